# revision 9
# baseline (speedup 1.0000x reference)
"""Bass/Tile Trainium2 kernel for dense causal multi-head attention.

Problem: x[2,2048,1024] -> qkv (w_qkv [3072,1024]) -> 16-head causal
attention -> out proj (w_proj [1024,1024], b_proj) -> [2,2048,1024].

Sharding over 8 NeuronCores: data-parallel over batch (2) x
tensor-parallel over heads (4 groups of 4 heads). Each core computes its
768-row slice of the qkv projection, causal attention for its 4 heads,
and a partial output projection over its 256 head-dim columns. The
all-reduce after proj is realized host-side at gather time (sum of 4
partials per batch) together with the bias add.

On-core layout: activations kept transposed ([feature, seq]) so that
  * scores are computed directly as S^T = K_tile^T-stationary @ Q-moving
    (no P transposes anywhere),
  * softmax reduction over keys happens via a ones-column appended to V
    (denominator falls out of the same PE accumulation as attn@V),
  * head pairs sit at partition offsets 0/64 and their K=64 score
    matmuls run concurrently in different PE row groups.

All matmul operands are bf16 (PSUM accumulation stays fp32): on TRN2
hardware fp32/fp32r moving operands stream at 2 cycles/column while
bf16 streams at 1, so bf16 halves tensor-engine time (the bottleneck).
exp() is applied to the fp32 PSUM scores, so only the bf16 rounding of
inputs/weights/P/V (~0.4% each, mostly incoherent) reaches the output;
tolerance is 2e-2.

Scheduling: the kernel is one long pipeline against the ScalarE exp
stream (~1.1us per k-tile step). Phase 1a computes only the chunk-0/1
slices of pair 0's qkv so attention starts early; the rest of pair 0's
qkv + all of pair 1's run as fillers inside pair 0's attention, and the
output projection inside pair 1's. Chunk-boundary normalization (PSUM
accumulator evict, denominator broadcast, 1/d multiply) is deferred
into the next chunk's early steps so the PE queue never stalls behind
it.
"""

import sys
from contextlib import ExitStack

if "/opt/trn_rl_repo" not in sys.path:
    sys.path.insert(0, "/opt/trn_rl_repo")

import numpy as np
import ml_dtypes

import concourse.bass as bass
import concourse.tile as tile
from concourse import bacc, mybir
from concourse.bass_utils import run_bass_kernel_spmd

F32 = mybir.dt.float32
BF16 = mybir.dt.bfloat16
AF = mybir.ActivationFunctionType

B, N, C = 2, 2048, 1024
H_TOT, D = 16, 64
NCORES = 8
HPC = H_TOT // (NCORES // B)  # heads per core = 4
HD = HPC * D                  # 256 per-core head-dim columns
CT = C // 128                 # 8 contraction tiles
NT = N // 128                 # 16 seq tiles
QCH = N // 512                # 4 query chunks of 512
SCALE = float(D) ** -0.5


class Ctx:
    """Shared build state."""
    pass


def _gemm_units(g, w_tiles, col0, dest, dest_slice_of, mm_pool, mm_tag,
                mm_bufs, tr_pool, tr_tag, tr_bufs, evict_engine,
                nchs=range(QCH)):
    """Filler units for one [128-col j-tile] x N GEMM: per 512-query chunk,
    4 units of 2 accumulating matmuls + 1 evict unit (+ V transposes)."""
    nc = g.nc
    units = []
    for nch in nchs:
        cell = {}
        ns = slice(nch * 512, (nch + 1) * 512)

        def mk_mm(cts, nch=nch, ns=ns, cell=cell):
            def u():
                if "ps" not in cell:
                    cell["ps"] = mm_pool.tile([128, 512], F32, tag=mm_tag,
                                              bufs=mm_bufs, name="gps")
                for ct in cts:
                    nc.tensor.matmul(
                        cell["ps"][:],
                        w_tiles[ct][:, col0:col0 + 128],
                        g.xt[ct][:, ns],
                        start=(ct == 0), stop=(ct == CT - 1),
                    )
            return u

        def mk_evict(nch=nch, ns=ns, cell=cell):
            def u():
                if evict_engine == "act":
                    nc.scalar.activation(dest[:, ns], cell["ps"][:], AF.Copy)
                else:
                    nc.vector.tensor_copy(dest[:, ns], cell["ps"][:])
            return u

        units.append(mk_mm([0, 1]))
        units.append(mk_mm([2, 3]))
        units.append(mk_mm([4, 5]))
        units.append(mk_mm([6, 7]))
        units.append(mk_evict())
        if dest_slice_of is not None:
            hp = dest_slice_of
            for nt in range(4 * nch, 4 * nch + 4):
                def tr(nt=nt, hp=hp):
                    pst = tr_pool.tile([128, 128], BF16, tag=tr_tag,
                                       bufs=tr_bufs, name="pst")
                    nc.tensor.transpose(
                        pst[:], dest[:, nt * 128:(nt + 1) * 128], g.identity[:])
                    # [v_even | v_odd] -> cols {0:64, 65:129} of the pair tile
                    vd = g.v_sb[hp][:, nt, :]
                    nc.vector.tensor_copy(
                        vd.rearrange("p (b c) -> p b c", b=2)[:, :, 0:64],
                        pst[:].rearrange("p (b c) -> p b c", b=2))
                units.append(tr)
    return units


def _proj_units(g, qc, psum_pool, o_pool, evict_engine="dve"):
    """Filler units for the output projection of seq tiles in chunk qc."""
    nc = g.nc
    units = []
    for nt in range(4 * qc, 4 * qc + 4):
        cell = {}

        def mk_mm(ht, nt=nt, cell=cell):
            def u():
                if "ps" not in cell:
                    cell["ps"] = [psum_pool.tile([128, 512], F32, tag="pr",
                                                 bufs=2, name="pso")
                                  for _ in range(2)]
                for cok in range(2):
                    nc.tensor.matmul(
                        cell["ps"][cok][:],
                        g.yT[:, ht, nt * 128:(nt + 1) * 128],
                        g.wp[ht][:, cok * 512:(cok + 1) * 512],
                        start=(ht == 0), stop=(ht == 1),
                    )
            return u

        def mk_out(cok, nt=nt, cell=cell):
            def u():
                ot = o_pool.tile([128, 512], BF16, tag="ot", name="ot")
                eng = evict_engine
                if eng == "mixed":
                    eng = "act" if cok == 0 else "dve"
                if eng == "act":
                    nc.scalar.activation(ot[:], cell["ps"][cok][:], AF.Copy)
                else:
                    nc.vector.tensor_copy(ot[:], cell["ps"][cok][:])
                nc.sync.dma_start(
                    g.out_r[nt, :, cok * 512:(cok + 1) * 512], ot[:])
            return u

        units.extend([mk_mm(0), mk_mm(1), mk_out(0), mk_out(1)])
    return units


def _attention_pair(g, hp, q_t, k_t, fillers, s_ps, av_ps, bc_ps,
                    p_pool, r_pool, bc_pool, chunk_cb=None,
                    step_cb=None, pbc_tag="pbc", pbc_bufs=1,
                    mask_mode="dve", tail_warm=False):
    """Causal attention for head pair hp, popping filler units into the
    exp (ScalarE) shadow of each k-tile step.

    One flat software pipeline across all (chunk, k-tile) steps: the AV
    matmul for step i issues alongside the S matmuls of step i+2 even
    across a chunk boundary, so the exp stream never bubbles while a
    chunk's accumulators drain. Chunk-boundary normalization (evict,
    denominator broadcast, 1/d multiply) is deferred a few steps into
    the following chunk for the same reason."""
    nc = g.nc
    total_steps = sum(4 * (qc + 1) + 2 for qc in range(QCH))
    state = {"fi": 0, "step": 0}

    def pop(nsteps):
        state["step"] += nsteps
        left = total_steps - state["step"]
        avail = len(fillers) - state["fi"]
        want = avail if left <= 0 else -(-avail // (left + 1)) * nsteps
        for _ in range(min(want, avail)):
            fillers[state["fi"]]()
            state["fi"] += 1

    def make_pts(qc, kt, qs):
        # both heads' S^T tiles into one 2-bank PSUM tile -> a single
        # [128,1024] exp (amortizes the ScalarE fixed overhead)
        diag = kt >= 4 * qc
        ps = s_ps.tile([128, 2, 512], F32, tag="s", bufs=2, name="pss")
        for po in range(2):
            o = 64 * po
            nc.tensor.matmul(
                ps[:, po, :],
                k_t[o:o + 64, kt * 128:(kt + 1) * 128],
                q_t[o:o + 64, qs],
                start=True, stop=not (diag and mask_mode == "pe"),
            )
        if diag and mask_mode == "pe":
            # additive causal mask: S += I @ (-1e30 triangle) in PSUM, so
            # the DVE stays off the exp->AV critical path
            for po in range(2):
                nc.tensor.matmul(
                    ps[:, po, :], g.identity[:],
                    g.maskb[:, kt - 4 * qc, :],
                    start=False, stop=True,
                )
        ptb = p_pool.tile([128, 2, 512], BF16, tag="pt", name="pt")
        nc.scalar.activation(ptb[:], ps[:], AF.Exp, scale=SCALE)
        pts = [ptb[:, 0, :], ptb[:, 1, :]]
        if diag and mask_mode != "pe":
            # one wide bf16 multiply masks both heads' tiles at 2x DVE rate
            nc.vector.tensor_mul(ptb[:], ptb[:],
                                 g.masks[:, kt - 4 * qc, :, :])
        return pts

    def mk_norm(po, av, qs):
        def f():
            pbc = bc_ps.tile([64, 512], F32, tag=pbc_tag, bufs=pbc_bufs,
                             name="pbc")
            nc.tensor.matmul(pbc[:], g.ones64[64:65, :], av[64:65, :],
                             start=True, stop=True)
            bc = bc_pool.tile([64, 512], F32, tag="bc", name="bc")
            nc.vector.reciprocal_approx_fast(bc[:], pbc[:])
            nc.vector.tensor_mul(
                g.yT[64 * po:64 * po + 64, hp, qs], av[0:64, :], bc[:])
        return f

    def warm_mm(n):
        wps = av_ps.tile([128, 512], F32, tag="av0", bufs=1, name="warm2")
        for _ in range(n):
            nc.tensor.matmul(wps[:], g.identity[:], g.wtmp[:],
                             start=True, stop=True)

    steps = [(qc, kt) for qc in range(QCH) for kt in range(4 * (qc + 1))]
    pend = []      # S/exp steps awaiting their AV (global lag of 2)
    pending = []   # deferred normalize / chunk_cb closures
    pavs = {}      # live AV accumulators by chunk
    for i in range(len(steps) + 2):
        if i < len(steps):
            qc, kt = steps[i]
            qs = slice(qc * 512, (qc + 1) * 512)
            pend.append((qc, kt, make_pts(qc, kt, qs)))
            if step_cb is not None and kt < 4:
                step_cb(qc, kt)
            if pending and 2 <= kt <= 4:
                pending.pop(0)()
        if len(pend) > 2 or (i >= len(steps) and pend):
            aqc, akt, pts = pend.pop(0)
            ankt = 4 * (aqc + 1)
            if akt == 0:
                # allocate here (not at S time) so the previous chunk's
                # accumulator evict is already issued -> clean WAR rotation
                pavs[aqc] = [av_ps.tile([65, 512], F32, tag=f"av{po}",
                                        bufs=1, name=f"pav{po}")
                             for po in range(2)]
            for po in range(2):
                nc.tensor.matmul(
                    pavs[aqc][po][:],
                    g.v_sb[hp][:, akt, 65 * po:65 * po + 65],
                    pts[po],
                    start=(akt == 0), stop=(akt == ankt - 1),
                )
            if akt == ankt - 1:
                # chunk fully accumulated: evict now (frees PSUM), defer
                # the denominator broadcast + normalize
                aqs = slice(aqc * 512, (aqc + 1) * 512)
                avs = []
                for po in range(2):
                    av = r_pool.tile([65, 512], BF16, tag="avsb", name="avsb")
                    nc.vector.tensor_copy(av[:], pavs[aqc][po][:])
                    avs.append(av)
                del pavs[aqc]
                pending.append(mk_norm(0, avs[0], aqs))
                pending.append(mk_norm(1, avs[1], aqs))
                if chunk_cb is not None:
                    pending.append(lambda aqc=aqc: chunk_cb(aqc))
        if i < len(steps):
            qc, kt = steps[i]
            pop(2 if kt < 2 else 1)
    while pending:
        if tail_warm:
            # dummy matmuls keep the HAM clock governor at full rate while
            # the DVE normalize chain runs, so the projection tail is warm
            warm_mm(3)
        pending.pop(0)()
    cnt = 0
    while state["fi"] < len(fillers):
        if tail_warm and cnt % 2 == 0:
            warm_mm(1)
        cnt += 1
        fillers[state["fi"]]()
        state["fi"] += 1


def build_nc():
    nc = bacc.Bacc("TRN2", target_bir_lowering=False, debug=False)
    xT = nc.dram_tensor("xT", [C, N], BF16, kind="ExternalInput").ap()
    wqkvT = nc.dram_tensor("wqkvT", [C, 3 * HD], BF16, kind="ExternalInput").ap()
    wpT = nc.dram_tensor("wpT", [HD, C], BF16, kind="ExternalInput").ap()
    identD = nc.dram_tensor("ident", [128, 128], BF16, kind="ExternalInput").ap()
    maskmD = nc.dram_tensor("maskm", [128, 4096], BF16, kind="ExternalInput").ap()
    maskbD = nc.dram_tensor("maskb", [128, 2048], BF16, kind="ExternalInput").ap()
    out = nc.dram_tensor("out", [N, C], BF16, kind="ExternalOutput").ap()

    xT_r = xT.rearrange("(ct p) n -> ct p n", p=128)
    wq_r = wqkvT.rearrange("(ct p) j -> ct p j", p=128)
    wp_r = wpT.rearrange("(ht p) co -> ht p co", p=128)

    g = Ctx()
    g.nc = nc
    g.out_r = out.rearrange("(nt p) co -> nt p co", p=128)

    with tile.TileContext(nc) as tc, ExitStack() as ctx:
        const = ctx.enter_context(tc.tile_pool(name="const", bufs=1))
        qkv_pool = ctx.enter_context(tc.tile_pool(name="qkv", bufs=1))
        yT_pool = ctx.enter_context(tc.tile_pool(name="yT", bufs=1))
        v_pool = ctx.enter_context(tc.tile_pool(name="v", bufs=1))
        mask_pool = ctx.enter_context(tc.tile_pool(name="mask", bufs=1))

        g.identity = const.tile([128, 128], BF16, tag="id")
        nc.sync.dma_start(g.identity[:], identD)
        ones64f = const.tile([128, 64], F32, tag="ones64f")
        nc.vector.memset(ones64f[:], 1.0)
        g.ones64 = const.tile([128, 64], BF16, tag="ones64")
        nc.vector.tensor_copy(g.ones64[:], ones64f[:])
        g.wtmp = const.tile([128, 512], BF16, tag="wtmp")
        nc.vector.memset(g.wtmp[:], 0.0)

        # q/k tiles per pair, [d-of-pair(128), N]
        q_t = [qkv_pool.tile([128, N], BF16, tag=f"q{hp}", name=f"qT{hp}")
               for hp in range(2)]
        k_t = [qkv_pool.tile([128, N], BF16, tag=f"k{hp}", name=f"kT{hp}")
               for hp in range(2)]
        g.yT = yT_pool.tile([128, 2, N], BF16, tag="yT")
        # V per pair: [k-partition, kt, 130] = [v_even |1| v_odd |1];
        # col 64/129 = ones (softmax denominator row of the AV matmul).
        g.v_sb = [v_pool.tile([128, NT, 130], BF16, tag=f"v{hp}",
                              name=f"v{hp}") for hp in range(2)]
        # Causal masks for the 4 diagonal positions of a 512-query chunk:
        # multiplicative (duplicated for both heads, one wide tensor_mul
        # masks a full [128, 2, 512] P tile) and additive (-1e30), both
        # prepared host-side so no gpsimd work gates the pipeline.
        g.masks = mask_pool.tile([128, 4, 2, 512], BF16, tag="mask")
        g.maskb = mask_pool.tile([128, 4, 512], BF16, tag="maskb")
        onescol = mask_pool.tile([128, NT], F32, tag="onescol")
        nc.vector.memset(onescol[:], 1.0)
        for hp in range(2):
            nc.vector.tensor_copy(g.v_sb[hp][:, :, 64], onescol[:])
            nc.vector.tensor_copy(g.v_sb[hp][:, :, 129], onescol[:])

        wp_pool = ctx.enter_context(tc.tile_pool(name="wp", bufs=1))
        g.wp = [wp_pool.tile([128, C], BF16, tag=f"wp{ht}", name=f"wp{ht}")
                for ht in range(2)]

        with tc.tile_pool(name="vt1", bufs=1) as vt1_pool:
            vt1 = vt1_pool.tile([128, N], BF16, tag="vt1")

            with tc.tile_pool(name="x", bufs=1) as x_pool, \
                 tc.tile_pool(name="wb", bufs=1) as wb_pool:
                xt_big = x_pool.tile([128, CT, N], BF16, tag="x",
                                     name="xt_big")
                g.xt = [xt_big[:, ct, :] for ct in range(CT)]
                wb_big = wb_pool.tile([128, CT, 384], BF16, tag="wb",
                                      name="wb_big")
                wb = [wb_big[:, ct, :] for ct in range(CT)]

                # ---- Phase 1a: pair-0 qkv, chunk-major so the PE
                # stream paces exactly behind the x chunk DMAs ----
                with tc.tile_pool(name="wa", bufs=1) as wa_pool, \
                     tc.tile_pool(name="vt0", bufs=1) as vt0_pool, \
                     tc.tile_pool(name="warm", bufs=1, space="PSUM") as wm_ps, \
                     tc.tile_pool(name="mmps", bufs=3, space="PSUM") as mm_ps:
                    wa_big = wa_pool.tile([128, CT, 384], BF16, tag="wa",
                                          name="wa_big")
                    wa = [wa_big[:, ct, :] for ct in range(CT)]
                    vt0 = vt0_pool.tile([128, N], BF16, tag="vt0")
                    # HAM warm-up: N=512 dummy matmuls keep the PE busy
                    # through the input-DMA window so the clock governor
                    # reaches full rate before the real qkv stream starts.
                    wps = wm_ps.tile([128, 512], F32, tag="warm", name="wps")
                    for _ in range(9):
                        nc.tensor.matmul(wps[:], g.identity[:], g.wtmp[:],
                                         start=True, stop=True)
                    wq_p = wq_r.rearrange("ct p j -> p ct j")
                    xT_p = xT_r.rearrange("ct p n -> p ct n")
                    # consolidated loads, ordered by first use; pair-1
                    # weights (wb) are only needed in phase 2a, so they
                    # load last and never stall the pair-0 stream.
                    nc.sync.dma_start(wa_big[:, :, 0:128], wq_p[:, :, 0:128])
                    nc.sync.dma_start(xt_big[:, 0:4, 0:512], xT_p[:, 0:4, 0:512])
                    nc.sync.dma_start(xt_big[:, 4:8, 0:512], xT_p[:, 4:8, 0:512])
                    nc.sync.dma_start(wa_big[:, :, 128:384],
                                      wq_p[:, :, 128:384])
                    nc.sync.dma_start(
                        g.masks[:],
                        maskmD.rearrange("p (a b f) -> p a b f", a=4, b=2))
                    for nch in range(1, QCH):
                        nc.sync.dma_start(
                            xt_big[:, :, nch * 512:(nch + 1) * 512],
                            xT_p[:, :, nch * 512:(nch + 1) * 512])
                    for ht in range(2):
                        nc.sync.dma_start(g.wp[ht][:], wp_r[ht])
                    nc.sync.dma_start(
                        g.maskb[:],
                        maskbD.rearrange("p (a f) -> p a f", a=4))
                    nc.sync.dma_start(wb_big[:], wq_p[:, :, 384:768])
                    for nch in range(QCH):
                        for col0, dest, dsl in ((0, q_t[0], None),
                                                (128, k_t[0], None),
                                                (256, vt0, 0)):
                            for u in _gemm_units(g, wa, col0, dest, dsl,
                                                 mm_ps, "mm", 4, mm_ps,
                                                 "tr1a", 3, "act",
                                                 nchs=[nch]):
                                u()

                # ---- Phase 2a: pair-0 attn; rest of pair-0 qkv + pair-1
                # qkv in the exp shadow ----
                with tc.tile_pool(name="p", bufs=6) as p_pool, \
                     tc.tile_pool(name="avsb", bufs=4) as r_pool, \
                     tc.tile_pool(name="bcast", bufs=2) as bc_pool, \
                     tc.tile_pool(name="sps", bufs=3, space="PSUM") as s_ps, \
                     tc.tile_pool(name="avps", bufs=1, space="PSUM") as av_ps, \
                     tc.tile_pool(name="bcps", bufs=1, space="PSUM") as bc_ps:
                    fillers = []
                    fillers += _gemm_units(g, wb, 0, q_t[1], None,
                                           bc_ps, "mm", 1, None, "", 0, "dve")
                    fillers += _gemm_units(g, wb, 128, k_t[1], None,
                                           bc_ps, "mm", 1, None, "", 0, "dve")
                    fillers += _gemm_units(g, wb, 256, vt1, None,
                                           bc_ps, "mm", 1, None, "", 0, "dve")
                    _attention_pair(g, 0, q_t[0], k_t[0], fillers,
                                    s_ps, av_ps, bc_ps, p_pool, r_pool,
                                    bc_pool)

            # ---- Phase 2b: pair-1 attention; V1 transposes + projection
            # in the exp shadow ----
            with tc.tile_pool(name="o", bufs=6) as o_pool, \
                 tc.tile_pool(name="p2", bufs=6) as p_pool, \
                 tc.tile_pool(name="avsb2", bufs=4) as r_pool, \
                 tc.tile_pool(name="bcast2", bufs=2) as bc_pool, \
                 tc.tile_pool(name="sps2", bufs=3, space="PSUM") as s_ps, \
                 tc.tile_pool(name="avps2", bufs=1, space="PSUM") as av_ps, \
                 tc.tile_pool(name="prps", bufs=1, space="PSUM") as pr_ps:
                fillers = []

                def step_cb(qc, kt):
                    # V1 transpose for the new k-tile this chunk will touch
                    # (must precede the AV matmul that reads v_sb[1]; AV
                    # for tile 4qc+kt runs 2+ steps later).
                    nt = 4 * qc + kt
                    pst = pr_ps.tile([128, 128], BF16, tag="pr", bufs=2,
                                     name="pst")
                    nc.tensor.transpose(
                        pst[:], vt1[:, nt * 128:(nt + 1) * 128],
                        g.identity[:])
                    vd = g.v_sb[1][:, nt, :]
                    nc.vector.tensor_copy(
                        vd.rearrange("p (b c) -> p b c", b=2)[:, :, 0:64],
                        pst[:].rearrange("p (b c) -> p b c", b=2))

                def chunk_cb(qc):
                    fillers.extend(_proj_units(
                        g, qc, pr_ps, o_pool,
                        evict_engine=("mixed" if qc == QCH - 1 else "dve")))

                _attention_pair(g, 1, q_t[1], k_t[1], fillers,
                                s_ps, av_ps, s_ps, p_pool, r_pool, bc_pool,
                                chunk_cb=chunk_cb, step_cb=step_cb,
                                pbc_tag="s", pbc_bufs=2, mask_mode="pe",
                                tail_warm=True)

    nc.compile()
    return nc


_NC = None


def _get_nc():
    global _NC
    if _NC is None:
        _NC = build_nc()
    return _NC


def make_in_maps(x, w_qkv, w_proj):
    x = np.asarray(x, dtype=np.float32)
    w_qkv = np.asarray(w_qkv, dtype=np.float32)
    w_proj = np.asarray(w_proj, dtype=np.float32)
    bf = ml_dtypes.bfloat16
    xT = [np.ascontiguousarray(x[b].T).astype(bf) for b in range(B)]
    ident = np.eye(128, dtype=bf)
    f = np.arange(512)[None, :]
    p = np.arange(128)[:, None]
    keep = np.stack([(f - p - 128 * r) >= 0 for r in range(4)], axis=1)
    maskm = np.repeat(keep.astype(bf)[:, :, None, :], 2, axis=2).reshape(128, 4096)
    maskm = np.ascontiguousarray(maskm)
    maskb = np.where(keep, 0.0, -1e30).astype(bf).reshape(128, 2048)
    maskb = np.ascontiguousarray(maskb)
    in_maps = []
    for c in range(NCORES):
        b, grp = divmod(c, NCORES // B)
        # pair-major row order: [q01 | k01 | v01 | q23 | k23 | v23]
        rows = []
        for hp in range(2):
            for s in range(3):  # q, k, v blocks of w_qkv
                base = s * C + grp * HD + hp * 2 * D
                rows.append(np.arange(base, base + 2 * D))
        rows = np.concatenate(rows)
        wqkvT = np.ascontiguousarray(w_qkv[rows, :].T).astype(bf)
        wpT = np.ascontiguousarray(w_proj[:, grp * HD:(grp + 1) * HD].T).astype(bf)
        in_maps.append({"xT": xT[b], "wqkvT": wqkvT, "wpT": wpT,
                        "ident": ident, "maskm": maskm, "maskb": maskb})
    return in_maps


def assemble(results, b_proj):
    b_proj = np.asarray(b_proj, dtype=np.float32)
    out = np.zeros((B, N, C), dtype=np.float32)
    for c in range(NCORES):
        b = c // (NCORES // B)
        out[b] += results[c]["out"].astype(np.float32)
    out += b_proj[None, None, :]
    return out


def kernel(x, w_qkv, w_proj, b_proj):
    nc = _get_nc()
    in_maps = make_in_maps(x, w_qkv, w_proj)
    res = run_bass_kernel_spmd(nc, in_maps, core_ids=list(range(NCORES)))
    return assemble(res.results, b_proj)


# revision 13
# speedup vs baseline: 1.0240x; 1.0240x over previous
"""Bass/Tile Trainium2 kernel for dense causal multi-head attention.

Problem: x[2,2048,1024] -> qkv (w_qkv [3072,1024]) -> 16-head causal
attention -> out proj (w_proj [1024,1024], b_proj) -> [2,2048,1024].

Sharding over 8 NeuronCores: data-parallel over batch (2) x
tensor-parallel over heads (4 groups of 4 heads). Each core computes its
768-row slice of the qkv projection, causal attention for its 4 heads,
and a partial output projection over its 256 head-dim columns. The
all-reduce after proj is realized host-side at gather time (sum of 4
partials per batch) together with the bias add.

On-core layout: activations kept transposed ([feature, seq]) so that
  * scores are computed directly as S^T = K_tile^T-stationary @ Q-moving
    (no P transposes anywhere),
  * softmax reduction over keys happens via a ones-column appended to V
    (denominator falls out of the same PE accumulation as attn@V),
  * head pairs sit at partition offsets 0/64 and their K=64 score
    matmuls run concurrently in different PE row groups.

All matmul operands are bf16 (PSUM accumulation stays fp32): on TRN2
hardware fp32/fp32r moving operands stream at 2 cycles/column while
bf16 streams at 1, so bf16 halves tensor-engine time (the bottleneck).
exp() is applied to the fp32 PSUM scores, so only the bf16 rounding of
inputs/weights/P/V (~0.4% each, mostly incoherent) reaches the output;
tolerance is 2e-2.

Scheduling: the kernel is one long pipeline against the ScalarE exp
stream (~1.1us per k-tile step). Phase 1a computes only the chunk-0/1
slices of pair 0's qkv so attention starts early; the rest of pair 0's
qkv + all of pair 1's run as fillers inside pair 0's attention, and the
output projection inside pair 1's. Chunk-boundary normalization (PSUM
accumulator evict, denominator broadcast, 1/d multiply) is deferred
into the next chunk's early steps so the PE queue never stalls behind
it.
"""

import sys
from contextlib import ExitStack

if "/opt/trn_rl_repo" not in sys.path:
    sys.path.insert(0, "/opt/trn_rl_repo")

import numpy as np
import ml_dtypes

import concourse.bass as bass
import concourse.tile as tile
from concourse import bacc, mybir
from concourse.bass_utils import run_bass_kernel_spmd

F32 = mybir.dt.float32
BF16 = mybir.dt.bfloat16
AF = mybir.ActivationFunctionType

B, N, C = 2, 2048, 1024
H_TOT, D = 16, 64
NCORES = 8
HPC = H_TOT // (NCORES // B)  # heads per core = 4
HD = HPC * D                  # 256 per-core head-dim columns
CT = C // 128                 # 8 contraction tiles
NT = N // 128                 # 16 seq tiles
QCH = N // 512                # 4 query chunks of 512
SCALE = float(D) ** -0.5


class Ctx:
    """Shared build state."""
    pass


def _gemm_units(g, w_tiles, col0, dest, dest_slice_of, mm_pool, mm_tag,
                mm_bufs, tr_pool, tr_tag, tr_bufs, evict_engine,
                nchs=range(QCH)):
    """Filler units for one [128-col j-tile] x N GEMM: per 512-query chunk,
    4 units of 2 accumulating matmuls + 1 evict unit (+ V transposes)."""
    nc = g.nc
    units = []
    for nch in nchs:
        cell = {}
        ns = slice(nch * 512, (nch + 1) * 512)

        def mk_mm(cts, nch=nch, ns=ns, cell=cell):
            def u():
                if "ps" not in cell:
                    cell["ps"] = mm_pool.tile([128, 512], F32, tag=mm_tag,
                                              bufs=mm_bufs, name="gps")
                for ct in cts:
                    nc.tensor.matmul(
                        cell["ps"][:],
                        w_tiles[ct][:, col0:col0 + 128],
                        g.xt[ct][:, ns],
                        start=(ct == 0), stop=(ct == CT - 1),
                    )
            return u

        def mk_evict(nch=nch, ns=ns, cell=cell):
            def u():
                if evict_engine == "act":
                    nc.scalar.activation(dest[:, ns], cell["ps"][:], AF.Copy)
                else:
                    nc.vector.tensor_copy(dest[:, ns], cell["ps"][:])
            return u

        units.append(mk_mm([0, 1]))
        units.append(mk_mm([2, 3]))
        units.append(mk_mm([4, 5]))
        units.append(mk_mm([6, 7]))
        units.append(mk_evict())
        if dest_slice_of is not None:
            hp = dest_slice_of
            for nt in range(4 * nch, 4 * nch + 4):
                def tr(nt=nt, hp=hp):
                    pst = tr_pool.tile([128, 128], BF16, tag=tr_tag,
                                       bufs=tr_bufs, name="pst")
                    nc.tensor.transpose(
                        pst[:], dest[:, nt * 128:(nt + 1) * 128], g.identity[:])
                    # [v_even | v_odd] -> cols {0:64, 65:129} of the pair tile
                    vd = g.v_sb[hp][:, nt, :]
                    nc.vector.tensor_copy(
                        vd.rearrange("p (b c) -> p b c", b=2)[:, :, 0:64],
                        pst[:].rearrange("p (b c) -> p b c", b=2))
                units.append(tr)
    return units


def _proj_units(g, qc, psum_pool, o_pool, evict_engine="dve"):
    """Filler units for the output projection of seq tiles in chunk qc."""
    nc = g.nc
    units = []
    for nt in range(4 * qc, 4 * qc + 4):
        cell = {}

        def mk_mm(ht, nt=nt, cell=cell):
            def u():
                if "ps" not in cell:
                    cell["ps"] = [psum_pool.tile([128, 512], F32, tag="pr",
                                                 bufs=2, name="pso")
                                  for _ in range(2)]
                for cok in range(2):
                    nc.tensor.matmul(
                        cell["ps"][cok][:],
                        g.yT[:, ht, nt * 128:(nt + 1) * 128],
                        g.wp[ht][:, cok * 512:(cok + 1) * 512],
                        start=(ht == 0), stop=(ht == 1),
                    )
            return u

        def mk_out(cok, nt=nt, cell=cell):
            def u():
                ot = o_pool.tile([128, 512], BF16, tag="ot", name="ot")
                eng = evict_engine
                if eng == "mixed":
                    eng = "act" if cok == 0 else "dve"
                if eng == "act":
                    nc.scalar.activation(ot[:], cell["ps"][cok][:], AF.Copy)
                else:
                    nc.vector.tensor_copy(ot[:], cell["ps"][cok][:])
                nc.sync.dma_start(
                    g.out_r[nt, :, cok * 512:(cok + 1) * 512], ot[:])
            return u

        units.extend([mk_mm(0), mk_mm(1), mk_out(0), mk_out(1)])
    return units


def _attention_pair(g, hp, q_t, k_t, fillers, s_ps, av_ps, bc_ps,
                    p_pool, r_pool, bc_pool, chunk_cb=None,
                    step_cb=None, pbc_tag="pbc", pbc_bufs=1,
                    mask_mode="dve", tail_warm=False):
    """Causal attention for head pair hp, popping filler units into the
    exp (ScalarE) shadow of each k-tile step.

    One flat software pipeline across all (chunk, k-tile) steps: the AV
    matmul for step i issues alongside the S matmuls of step i+2 even
    across a chunk boundary, so the exp stream never bubbles while a
    chunk's accumulators drain. Chunk-boundary normalization (evict,
    denominator broadcast, 1/d multiply) is deferred a few steps into
    the following chunk for the same reason."""
    nc = g.nc
    total_steps = sum(4 * (qc + 1) + 2 for qc in range(QCH))
    state = {"fi": 0, "step": 0}

    def pop(nsteps):
        state["step"] += nsteps
        left = total_steps - state["step"]
        avail = len(fillers) - state["fi"]
        want = avail if left <= 0 else -(-avail // (left + 1)) * nsteps
        for _ in range(min(want, avail)):
            fillers[state["fi"]]()
            state["fi"] += 1

    def make_pts(qc, kt, qs):
        # both heads' S^T tiles into one 2-bank PSUM tile -> a single
        # [128,1024] exp (amortizes the ScalarE fixed overhead)
        diag = kt >= 4 * qc
        ps = s_ps.tile([128, 2, 512], F32, tag="s", bufs=2, name="pss")
        for po in range(2):
            o = 64 * po
            nc.tensor.matmul(
                ps[:, po, :],
                k_t[o:o + 64, kt * 128:(kt + 1) * 128],
                q_t[o:o + 64, qs],
                start=True, stop=not (diag and mask_mode == "pe"),
            )
        if diag and mask_mode == "pe":
            # additive causal mask: S += I @ (-1e30 triangle) in PSUM, so
            # the DVE stays off the exp->AV critical path
            for po in range(2):
                nc.tensor.matmul(
                    ps[:, po, :], g.identity[:],
                    g.maskb[:, kt - 4 * qc, :],
                    start=False, stop=True,
                )
        ptb = p_pool.tile([128, 2, 512], BF16, tag="pt", name="pt")
        nc.scalar.activation(ptb[:], ps[:], AF.Exp, scale=SCALE)
        pts = [ptb[:, 0, :], ptb[:, 1, :]]
        if diag and mask_mode != "pe":
            # one wide bf16 multiply masks both heads' tiles at 2x DVE rate
            nc.vector.tensor_mul(ptb[:], ptb[:],
                                 g.masks[:, kt - 4 * qc, :, :])
        return pts

    def mk_norm(po, av, qs):
        def f():
            # hop the denominator row to partition 0 (tiny SBUF->SBUF
            # DMA), broadcast it across partitions on the (idle) GPSIMD
            # instead of a PE rank-1 matmul, then reciprocal + scale
            dn = bc_pool.tile([1, 512], F32, tag="dn", name="dn")
            nc.sync.dma_start(dn[0:1, :], av[64:65, :])
            bcd = bc_pool.tile([64, 512], F32, tag="bcd", name="bcd")
            nc.gpsimd.partition_broadcast(bcd[:], dn[0:1, :])
            bc = bc_pool.tile([64, 512], F32, tag="bc", name="bc")
            nc.vector.reciprocal_approx_fast(bc[:], bcd[:])
            nc.vector.tensor_mul(
                g.yT[64 * po:64 * po + 64, hp, qs], av[0:64, :], bc[:])
        return f

    def warm_mm(n):
        wps = av_ps.tile([128, 512], F32, tag="av0", bufs=1, name="warm2")
        for _ in range(n):
            nc.tensor.matmul(wps[:], g.identity[:], g.wtmp[:],
                             start=True, stop=True)

    steps = [(qc, kt) for qc in range(QCH) for kt in range(4 * (qc + 1))]
    pend = []      # S/exp steps awaiting their AV (global lag of 2)
    pending = []   # deferred normalize / chunk_cb closures
    pavs = {}      # live AV accumulators by chunk
    for i in range(len(steps) + 2):
        if i < len(steps):
            qc, kt = steps[i]
            qs = slice(qc * 512, (qc + 1) * 512)
            pend.append((qc, kt, make_pts(qc, kt, qs)))
            if step_cb is not None and kt < 4:
                step_cb(qc, kt)
            if pending and 2 <= kt <= 4:
                pending.pop(0)()
        if len(pend) > 2 or (i >= len(steps) and pend):
            aqc, akt, pts = pend.pop(0)
            ankt = 4 * (aqc + 1)
            if akt == 0:
                # allocate here (not at S time) so the previous chunk's
                # accumulator evict is already issued -> clean WAR rotation
                pavs[aqc] = [av_ps.tile([65, 512], F32, tag=f"av{po}",
                                        bufs=1, name=f"pav{po}")
                             for po in range(2)]
            for po in range(2):
                nc.tensor.matmul(
                    pavs[aqc][po][:],
                    g.v_sb[hp][:, akt, 65 * po:65 * po + 65],
                    pts[po],
                    start=(akt == 0), stop=(akt == ankt - 1),
                )
            if akt == ankt - 1:
                # chunk fully accumulated: evict now (frees PSUM), defer
                # the denominator broadcast + normalize
                aqs = slice(aqc * 512, (aqc + 1) * 512)
                avs = []
                for po in range(2):
                    av = r_pool.tile([65, 512], F32, tag="avsb", name="avsb")
                    nc.vector.tensor_copy(av[:], pavs[aqc][po][:])
                    avs.append(av)
                del pavs[aqc]
                pending.append(mk_norm(0, avs[0], aqs))
                pending.append(mk_norm(1, avs[1], aqs))
                if chunk_cb is not None:
                    pending.append(lambda aqc=aqc: chunk_cb(aqc))
        if i < len(steps):
            qc, kt = steps[i]
            nkt = 4 * (qc + 1)
            # quota nkt+2 per chunk, spread over kt in [2, nkt-1): the
            # boundary steps carry the cross-chunk AV drain + new S pair,
            # so fillers there would bubble the exp stream
            if 2 <= kt < nkt - 1:
                mid = nkt - 3
                quota = nkt + 2
                j = kt - 2
                amt = (quota * (j + 1)) // mid - (quota * j) // mid
                pop(amt)
    while pending:
        if tail_warm:
            # dummy matmuls keep the HAM clock governor at full rate while
            # the DVE normalize chain runs, so the projection tail is warm
            warm_mm(3)
        pending.pop(0)()
    cnt = 0
    while state["fi"] < len(fillers):
        if tail_warm and cnt % 2 == 0:
            warm_mm(1)
        cnt += 1
        fillers[state["fi"]]()
        state["fi"] += 1


def build_nc():
    nc = bacc.Bacc("TRN2", target_bir_lowering=False, debug=False)
    xT = nc.dram_tensor("xT", [C, N], BF16, kind="ExternalInput").ap()
    wqkvT = nc.dram_tensor("wqkvT", [C, 3 * HD], BF16, kind="ExternalInput").ap()
    wpT = nc.dram_tensor("wpT", [HD, C], BF16, kind="ExternalInput").ap()
    identD = nc.dram_tensor("ident", [128, 128], BF16, kind="ExternalInput").ap()
    maskmD = nc.dram_tensor("maskm", [128, 4096], BF16, kind="ExternalInput").ap()
    maskbD = nc.dram_tensor("maskb", [128, 2048], BF16, kind="ExternalInput").ap()
    out = nc.dram_tensor("out", [N, C], BF16, kind="ExternalOutput").ap()

    xT_r = xT.rearrange("(ct p) n -> ct p n", p=128)
    wq_r = wqkvT.rearrange("(ct p) j -> ct p j", p=128)
    wp_r = wpT.rearrange("(ht p) co -> ht p co", p=128)

    g = Ctx()
    g.nc = nc
    g.out_r = out.rearrange("(nt p) co -> nt p co", p=128)

    with tile.TileContext(nc) as tc, ExitStack() as ctx:
        const = ctx.enter_context(tc.tile_pool(name="const", bufs=1))
        qkv_pool = ctx.enter_context(tc.tile_pool(name="qkv", bufs=1))
        yT_pool = ctx.enter_context(tc.tile_pool(name="yT", bufs=1))
        v_pool = ctx.enter_context(tc.tile_pool(name="v", bufs=1))
        mask_pool = ctx.enter_context(tc.tile_pool(name="mask", bufs=1))

        g.identity = const.tile([128, 128], BF16, tag="id")
        nc.sync.dma_start(g.identity[:], identD)
        ones64f = const.tile([128, 64], F32, tag="ones64f")
        nc.vector.memset(ones64f[:], 1.0)
        g.ones64 = const.tile([128, 64], BF16, tag="ones64")
        nc.vector.tensor_copy(g.ones64[:], ones64f[:])
        g.wtmp = const.tile([128, 512], BF16, tag="wtmp")
        nc.vector.memset(g.wtmp[:], 0.0)
        # dummy broadcast preloads the Q7 IRAM kernel (~6us) while the PE
        # is still in its warm-up window
        pbscr = const.tile([64, 64], F32, tag="pbscr")
        nc.gpsimd.partition_broadcast(pbscr[:], ones64f[0:1, :])

        # q/k tiles per pair, [d-of-pair(128), N]
        q_t = [qkv_pool.tile([128, N], BF16, tag=f"q{hp}", name=f"qT{hp}")
               for hp in range(2)]
        k_t = [qkv_pool.tile([128, N], BF16, tag=f"k{hp}", name=f"kT{hp}")
               for hp in range(2)]
        g.yT = yT_pool.tile([128, 2, N], BF16, tag="yT")
        # V per pair: [k-partition, kt, 130] = [v_even |1| v_odd |1];
        # col 64/129 = ones (softmax denominator row of the AV matmul).
        g.v_sb = [v_pool.tile([128, NT, 130], BF16, tag=f"v{hp}",
                              name=f"v{hp}") for hp in range(2)]
        # Causal masks for the 4 diagonal positions of a 512-query chunk:
        # multiplicative (duplicated for both heads, one wide tensor_mul
        # masks a full [128, 2, 512] P tile) and additive (-1e30), both
        # prepared host-side so no gpsimd work gates the pipeline.
        g.masks = mask_pool.tile([128, 4, 2, 512], BF16, tag="mask")
        g.maskb = mask_pool.tile([128, 4, 512], BF16, tag="maskb")
        onescol = mask_pool.tile([128, NT], F32, tag="onescol")
        nc.vector.memset(onescol[:], 1.0)
        for hp in range(2):
            nc.vector.tensor_copy(g.v_sb[hp][:, :, 64], onescol[:])
            nc.vector.tensor_copy(g.v_sb[hp][:, :, 129], onescol[:])

        wp_pool = ctx.enter_context(tc.tile_pool(name="wp", bufs=1))
        g.wp = [wp_pool.tile([128, C], BF16, tag=f"wp{ht}", name=f"wp{ht}")
                for ht in range(2)]

        with tc.tile_pool(name="vt1", bufs=1) as vt1_pool:
            vt1 = vt1_pool.tile([128, N], BF16, tag="vt1")

            with tc.tile_pool(name="x", bufs=1) as x_pool, \
                 tc.tile_pool(name="wb", bufs=1) as wb_pool:
                xt_big = x_pool.tile([128, CT, N], BF16, tag="x",
                                     name="xt_big")
                g.xt = [xt_big[:, ct, :] for ct in range(CT)]
                wb_big = wb_pool.tile([128, CT, 384], BF16, tag="wb",
                                      name="wb_big")
                wb = [wb_big[:, ct, :] for ct in range(CT)]

                # ---- Phase 1a: pair-0 qkv, chunk-major so the PE
                # stream paces exactly behind the x chunk DMAs ----
                with tc.tile_pool(name="wa", bufs=1) as wa_pool, \
                     tc.tile_pool(name="vt0", bufs=1) as vt0_pool, \
                     tc.tile_pool(name="warm", bufs=1, space="PSUM") as wm_ps, \
                     tc.tile_pool(name="mmps", bufs=3, space="PSUM") as mm_ps:
                    wa_big = wa_pool.tile([128, CT, 384], BF16, tag="wa",
                                          name="wa_big")
                    wa = [wa_big[:, ct, :] for ct in range(CT)]
                    vt0 = vt0_pool.tile([128, N], BF16, tag="vt0")
                    # HAM warm-up: N=512 dummy matmuls keep the PE busy
                    # through the input-DMA window so the clock governor
                    # reaches full rate before the real qkv stream starts.
                    wps = wm_ps.tile([128, 512], F32, tag="warm", name="wps")
                    for _ in range(9):
                        nc.tensor.matmul(wps[:], g.identity[:], g.wtmp[:],
                                         start=True, stop=True)
                    wq_p = wq_r.rearrange("ct p j -> p ct j")
                    xT_p = xT_r.rearrange("ct p n -> p ct n")
                    # consolidated loads, ordered by first use; pair-1
                    # weights (wb) are only needed in phase 2a, so they
                    # load last and never stall the pair-0 stream.
                    nc.sync.dma_start(wa_big[:, :, 0:128], wq_p[:, :, 0:128])
                    nc.sync.dma_start(xt_big[:, 0:4, 0:512], xT_p[:, 0:4, 0:512])
                    nc.sync.dma_start(xt_big[:, 4:8, 0:512], xT_p[:, 4:8, 0:512])
                    nc.sync.dma_start(wa_big[:, :, 128:384],
                                      wq_p[:, :, 128:384])
                    for nch in range(1, QCH):
                        nc.sync.dma_start(
                            xt_big[:, :, nch * 512:(nch + 1) * 512],
                            xT_p[:, :, nch * 512:(nch + 1) * 512])
                    for ht in range(2):
                        nc.sync.dma_start(g.wp[ht][:], wp_r[ht])
                    nc.sync.dma_start(
                        g.masks[:],
                        maskmD.rearrange("p (a b f) -> p a b f", a=4, b=2))
                    nc.sync.dma_start(
                        g.maskb[:],
                        maskbD.rearrange("p (a f) -> p a f", a=4))
                    nc.sync.dma_start(wb_big[:], wq_p[:, :, 384:768])
                    for nch in range(QCH):
                        for col0, dest, dsl in ((0, q_t[0], None),
                                                (128, k_t[0], None),
                                                (256, vt0, 0)):
                            for u in _gemm_units(g, wa, col0, dest, dsl,
                                                 mm_ps, "mm", 4, mm_ps,
                                                 "tr1a", 3, "act",
                                                 nchs=[nch]):
                                u()

                # ---- Phase 2a: pair-0 attn; rest of pair-0 qkv + pair-1
                # qkv in the exp shadow ----
                with tc.tile_pool(name="p", bufs=6) as p_pool, \
                     tc.tile_pool(name="avsb", bufs=4) as r_pool, \
                     tc.tile_pool(name="bcast", bufs=2) as bc_pool, \
                     tc.tile_pool(name="sps", bufs=3, space="PSUM") as s_ps, \
                     tc.tile_pool(name="avps", bufs=1, space="PSUM") as av_ps, \
                     tc.tile_pool(name="bcps", bufs=1, space="PSUM") as bc_ps:
                    fillers = []
                    fillers += _gemm_units(g, wb, 0, q_t[1], None,
                                           bc_ps, "mm", 1, None, "", 0, "dve")
                    fillers += _gemm_units(g, wb, 128, k_t[1], None,
                                           bc_ps, "mm", 1, None, "", 0, "dve")
                    fillers += _gemm_units(g, wb, 256, vt1, None,
                                           bc_ps, "mm", 1, None, "", 0, "dve")
                    _attention_pair(g, 0, q_t[0], k_t[0], fillers,
                                    s_ps, av_ps, bc_ps, p_pool, r_pool,
                                    bc_pool)

            # ---- Phase 2b: pair-1 attention; V1 transposes + projection
            # in the exp shadow ----
            with tc.tile_pool(name="o", bufs=6) as o_pool, \
                 tc.tile_pool(name="p2", bufs=6) as p_pool, \
                 tc.tile_pool(name="avsb2", bufs=4) as r_pool, \
                 tc.tile_pool(name="bcast2", bufs=2) as bc_pool, \
                 tc.tile_pool(name="sps2", bufs=3, space="PSUM") as s_ps, \
                 tc.tile_pool(name="avps2", bufs=1, space="PSUM") as av_ps, \
                 tc.tile_pool(name="prps", bufs=1, space="PSUM") as pr_ps:
                fillers = []

                def step_cb(qc, kt):
                    # V1 transpose for the new k-tile this chunk will touch
                    # (must precede the AV matmul that reads v_sb[1]; AV
                    # for tile 4qc+kt runs 2+ steps later).
                    nt = 4 * qc + kt
                    pst = pr_ps.tile([128, 128], BF16, tag="pr", bufs=2,
                                     name="pst")
                    nc.tensor.transpose(
                        pst[:], vt1[:, nt * 128:(nt + 1) * 128],
                        g.identity[:])
                    vd = g.v_sb[1][:, nt, :]
                    nc.vector.tensor_copy(
                        vd.rearrange("p (b c) -> p b c", b=2)[:, :, 0:64],
                        pst[:].rearrange("p (b c) -> p b c", b=2))

                def chunk_cb(qc):
                    fillers.extend(_proj_units(
                        g, qc, pr_ps, o_pool,
                        evict_engine=("mixed" if qc == QCH - 1 else "dve")))

                _attention_pair(g, 1, q_t[1], k_t[1], fillers,
                                s_ps, av_ps, s_ps, p_pool, r_pool, bc_pool,
                                chunk_cb=chunk_cb, step_cb=step_cb,
                                pbc_tag="s", pbc_bufs=2, mask_mode="pe",
                                tail_warm=True)

    nc.compile()
    return nc


_NC = None


def _get_nc():
    global _NC
    if _NC is None:
        _NC = build_nc()
    return _NC


def make_in_maps(x, w_qkv, w_proj):
    x = np.asarray(x, dtype=np.float32)
    w_qkv = np.asarray(w_qkv, dtype=np.float32)
    w_proj = np.asarray(w_proj, dtype=np.float32)
    bf = ml_dtypes.bfloat16
    xT = [np.ascontiguousarray(x[b].T).astype(bf) for b in range(B)]
    ident = np.eye(128, dtype=bf)
    f = np.arange(512)[None, :]
    p = np.arange(128)[:, None]
    keep = np.stack([(f - p - 128 * r) >= 0 for r in range(4)], axis=1)
    maskm = np.repeat(keep.astype(bf)[:, :, None, :], 2, axis=2).reshape(128, 4096)
    maskm = np.ascontiguousarray(maskm)
    maskb = np.where(keep, 0.0, -1e30).astype(bf).reshape(128, 2048)
    maskb = np.ascontiguousarray(maskb)
    in_maps = []
    for c in range(NCORES):
        b, grp = divmod(c, NCORES // B)
        # pair-major row order: [q01 | k01 | v01 | q23 | k23 | v23]
        rows = []
        for hp in range(2):
            for s in range(3):  # q, k, v blocks of w_qkv
                base = s * C + grp * HD + hp * 2 * D
                rows.append(np.arange(base, base + 2 * D))
        rows = np.concatenate(rows)
        wqkvT = np.ascontiguousarray(w_qkv[rows, :].T).astype(bf)
        wpT = np.ascontiguousarray(w_proj[:, grp * HD:(grp + 1) * HD].T).astype(bf)
        in_maps.append({"xT": xT[b], "wqkvT": wqkvT, "wpT": wpT,
                        "ident": ident, "maskm": maskm, "maskb": maskb})
    return in_maps


def assemble(results, b_proj):
    b_proj = np.asarray(b_proj, dtype=np.float32)
    out = np.zeros((B, N, C), dtype=np.float32)
    for c in range(NCORES):
        b = c // (NCORES // B)
        out[b] += results[c]["out"].astype(np.float32)
    out += b_proj[None, None, :]
    return out


def kernel(x, w_qkv, w_proj, b_proj):
    nc = _get_nc()
    in_maps = make_in_maps(x, w_qkv, w_proj)
    res = run_bass_kernel_spmd(nc, in_maps, core_ids=list(range(NCORES)))
    return assemble(res.results, b_proj)


# revision 14
# speedup vs baseline: 1.0558x; 1.0311x over previous
"""Bass/Tile Trainium2 kernel for dense causal multi-head attention.

Problem: x[2,2048,1024] -> qkv (w_qkv [3072,1024]) -> 16-head causal
attention -> out proj (w_proj [1024,1024], b_proj) -> [2,2048,1024].

Sharding over 8 NeuronCores: data-parallel over batch (2) x
tensor-parallel over heads (4 groups of 4 heads). Each core computes its
768-row slice of the qkv projection, causal attention for its 4 heads,
and a partial output projection over its 256 head-dim columns. The
all-reduce after proj is realized host-side at gather time (sum of 4
partials per batch) together with the bias add.

On-core layout: activations kept transposed ([feature, seq]) so that
  * scores are computed directly as S^T = K_tile^T-stationary @ Q-moving
    (no P transposes anywhere),
  * softmax reduction over keys happens via a ones-column appended to V
    (denominator falls out of the same PE accumulation as attn@V),
  * head pairs sit at partition offsets 0/64 and their K=64 score
    matmuls run concurrently in different PE row groups.

All matmul operands are bf16 (PSUM accumulation stays fp32): on TRN2
hardware fp32/fp32r moving operands stream at 2 cycles/column while
bf16 streams at 1, so bf16 halves tensor-engine time (the bottleneck).
exp() is applied to the fp32 PSUM scores, so only the bf16 rounding of
inputs/weights/P/V (~0.4% each, mostly incoherent) reaches the output;
tolerance is 2e-2.

Scheduling: the kernel is one long pipeline against the ScalarE exp
stream (~1.1us per k-tile step). Phase 1a computes only the chunk-0/1
slices of pair 0's qkv so attention starts early; the rest of pair 0's
qkv + all of pair 1's run as fillers inside pair 0's attention, and the
output projection inside pair 1's. Chunk-boundary normalization (PSUM
accumulator evict, denominator broadcast, 1/d multiply) is deferred
into the next chunk's early steps so the PE queue never stalls behind
it.
"""

import sys
from contextlib import ExitStack

if "/opt/trn_rl_repo" not in sys.path:
    sys.path.insert(0, "/opt/trn_rl_repo")

import numpy as np
import ml_dtypes

import concourse.bass as bass
import concourse.tile as tile
from concourse import bacc, mybir
from concourse.bass_utils import run_bass_kernel_spmd

F32 = mybir.dt.float32
BF16 = mybir.dt.bfloat16
AF = mybir.ActivationFunctionType

B, N, C = 2, 2048, 1024
H_TOT, D = 16, 64
NCORES = 8
HPC = H_TOT // (NCORES // B)  # heads per core = 4
HD = HPC * D                  # 256 per-core head-dim columns
CT = C // 128                 # 8 contraction tiles
NT = N // 128                 # 16 seq tiles
QCH = N // 512                # 4 query chunks of 512
SCALE = float(D) ** -0.5


class Ctx:
    """Shared build state."""
    pass


def _gemm_units(g, w_tiles, col0, dest, dest_slice_of, mm_pool, mm_tag,
                mm_bufs, tr_pool, tr_tag, tr_bufs, evict_engine,
                nchs=range(QCH)):
    """Filler units for one [128-col j-tile] x N GEMM: per 512-query chunk,
    4 units of 2 accumulating matmuls + 1 evict unit (+ V transposes)."""
    nc = g.nc
    units = []
    for nch in nchs:
        cell = {}
        ns = slice(nch * 512, (nch + 1) * 512)

        def mk_mm(cts, nch=nch, ns=ns, cell=cell):
            def u():
                if "ps" not in cell:
                    cell["ps"] = mm_pool.tile([128, 512], F32, tag=mm_tag,
                                              bufs=mm_bufs, name="gps")
                for ct in cts:
                    nc.tensor.matmul(
                        cell["ps"][:],
                        w_tiles[ct][:, col0:col0 + 128],
                        g.xt[ct][:, ns],
                        start=(ct == 0), stop=(ct == CT - 1),
                    )
            return u

        def mk_evict(nch=nch, ns=ns, cell=cell):
            def u():
                if evict_engine == "act":
                    nc.scalar.activation(dest[:, ns], cell["ps"][:], AF.Copy)
                else:
                    nc.vector.tensor_copy(dest[:, ns], cell["ps"][:])
            return u

        units.append(mk_mm([0, 1]))
        units.append(mk_mm([2, 3]))
        units.append(mk_mm([4, 5]))
        units.append(mk_mm([6, 7]))
        units.append(mk_evict())
        if dest_slice_of is not None:
            hp = dest_slice_of
            for nt in range(4 * nch, 4 * nch + 4):
                def tr(nt=nt, hp=hp):
                    pst = tr_pool.tile([128, 128], BF16, tag=tr_tag,
                                       bufs=tr_bufs, name="pst")
                    nc.tensor.transpose(
                        pst[:], dest[:, nt * 128:(nt + 1) * 128], g.identity[:])
                    # [v_even | v_odd] -> cols {0:64, 65:129} of the pair tile
                    vd = g.v_sb[hp][:, nt, :]
                    nc.vector.tensor_copy(
                        vd.rearrange("p (b c) -> p b c", b=2)[:, :, 0:64],
                        pst[:].rearrange("p (b c) -> p b c", b=2))
                units.append(tr)
    return units


def _proj_units(g, qc, psum_pool, o_pool, evict_engine="dve"):
    """Filler units for the output projection of seq tiles in chunk qc."""
    nc = g.nc
    units = []
    for nt in range(4 * qc, 4 * qc + 4):
        cell = {}

        def mk_mm(ht, nt=nt, cell=cell):
            def u():
                if "ps" not in cell:
                    cell["ps"] = [psum_pool.tile([128, 512], F32, tag="pr",
                                                 bufs=2, name="pso")
                                  for _ in range(2)]
                for cok in range(2):
                    nc.tensor.matmul(
                        cell["ps"][cok][:],
                        g.yT[:, ht, nt * 128:(nt + 1) * 128],
                        g.wp[ht][:, cok * 512:(cok + 1) * 512],
                        start=(ht == 0), stop=(ht == 1),
                    )
            return u

        def mk_out(cok, nt=nt, cell=cell):
            def u():
                ot = o_pool.tile([128, 512], BF16, tag="ot", name="ot")
                eng = evict_engine
                if eng == "mixed":
                    eng = "act" if cok == 0 else "dve"
                if eng == "act":
                    nc.scalar.activation(ot[:], cell["ps"][cok][:], AF.Copy)
                else:
                    nc.vector.tensor_copy(ot[:], cell["ps"][cok][:])
                nc.sync.dma_start(
                    g.out_r[nt, :, cok * 512:(cok + 1) * 512], ot[:])
            return u

        units.extend([mk_mm(0), mk_mm(1), mk_out(0), mk_out(1)])
    return units


def _attention_pair(g, hp, q_t, k_t, fillers, s_ps, av_ps, bc_ps,
                    p_pool, r_pool, bc_pool, chunk_cb=None,
                    step_cb=None, pbc_tag="pbc", pbc_bufs=1,
                    tail_warm=False):
    """Causal attention for head pair hp, popping filler units into the
    exp (ScalarE) shadow of each k-tile step.

    One flat software pipeline across all (chunk, k-tile) steps: the AV
    matmul for step i issues alongside the S matmuls of step i+2 even
    across a chunk boundary, so the exp stream never bubbles while a
    chunk's accumulators drain. Chunk-boundary normalization (evict,
    denominator broadcast, 1/d multiply) is deferred a few steps into
    the following chunk for the same reason."""
    nc = g.nc
    total_steps = sum(4 * (qc + 1) + 2 for qc in range(QCH))
    state = {"fi": 0, "step": 0}

    def pop(nsteps):
        state["step"] += nsteps
        left = total_steps - state["step"]
        avail = len(fillers) - state["fi"]
        want = avail if left <= 0 else -(-avail // (left + 1)) * nsteps
        for _ in range(min(want, avail)):
            fillers[state["fi"]]()
            state["fi"] += 1

    def make_pts(qc, kt, qs):
        # both heads' S^T tiles into one 2-bank PSUM tile -> a single
        # wide exp (amortizes the ScalarE fixed overhead). For diagonal
        # k-tiles only the causally-valid query strip [128r:] is
        # exponentiated; the fully-masked prefix is memset to zero and
        # the 128-wide triangle at the strip start gets a small bf16
        # multiply. This keeps ~30% of exp work off the ScalarE, which
        # paces the whole attention pipeline.
        diag = kt >= 4 * qc
        r = kt - 4 * qc if diag else 0
        ps = s_ps.tile([128, 2, 512], F32, tag="s", bufs=2, name="pss")
        for po in range(2):
            o = 64 * po
            nc.tensor.matmul(
                ps[:, po, :],
                k_t[o:o + 64, kt * 128:(kt + 1) * 128],
                q_t[o:o + 64, qs],
                start=True, stop=True,
            )
        ptb = p_pool.tile([128, 2, 512], BF16, tag="pt", name="pt")
        if r > 0:
            nc.vector.memset(ptb[:, :, 0:128 * r], 0.0)
        nc.scalar.activation(ptb[:, :, 128 * r:], ps[:, :, 128 * r:],
                             AF.Exp, scale=SCALE)
        if diag:
            sl = slice(128 * r, 128 * r + 128)
            nc.vector.tensor_mul(ptb[:, :, sl], ptb[:, :, sl], g.masks[:])
        return [ptb[:, 0, :], ptb[:, 1, :]]

    def mk_norm(po, av, qs):
        def f():
            # hop the denominator row to partition 0 (tiny SBUF->SBUF
            # DMA), broadcast it across partitions on the (idle) GPSIMD
            # instead of a PE rank-1 matmul, then reciprocal + scale
            dn = bc_pool.tile([1, 512], F32, tag="dn", name="dn")
            nc.sync.dma_start(dn[0:1, :], av[64:65, :])
            bcd = bc_pool.tile([64, 512], F32, tag="bcd", name="bcd")
            nc.gpsimd.partition_broadcast(bcd[:], dn[0:1, :])
            bc = bc_pool.tile([64, 512], F32, tag="bc", name="bc")
            nc.vector.reciprocal_approx_fast(bc[:], bcd[:])
            nc.vector.tensor_mul(
                g.yT[64 * po:64 * po + 64, hp, qs], av[0:64, :], bc[:])
        return f

    def warm_mm(n):
        wps = av_ps.tile([128, 512], F32, tag="av0", bufs=1, name="warm2")
        for _ in range(n):
            nc.tensor.matmul(wps[:], g.identity[:], g.wtmp[:],
                             start=True, stop=True)

    steps = [(qc, kt) for qc in range(QCH) for kt in range(4 * (qc + 1))]
    pend = []      # S/exp steps awaiting their AV (global lag of 2)
    pending = []   # deferred normalize / chunk_cb closures
    pavs = {}      # live AV accumulators by chunk
    for i in range(len(steps) + 2):
        if i < len(steps):
            qc, kt = steps[i]
            qs = slice(qc * 512, (qc + 1) * 512)
            pend.append((qc, kt, make_pts(qc, kt, qs)))
            if step_cb is not None and kt < 4:
                step_cb(qc, kt)
            if pending and 2 <= kt <= 4:
                pending.pop(0)()
        if len(pend) > 2 or (i >= len(steps) and pend):
            aqc, akt, pts = pend.pop(0)
            ankt = 4 * (aqc + 1)
            if akt == 0:
                # allocate here (not at S time) so the previous chunk's
                # accumulator evict is already issued -> clean WAR rotation
                pavs[aqc] = [av_ps.tile([65, 512], F32, tag=f"av{po}",
                                        bufs=1, name=f"pav{po}")
                             for po in range(2)]
            for po in range(2):
                nc.tensor.matmul(
                    pavs[aqc][po][:],
                    g.v_sb[hp][:, akt, 65 * po:65 * po + 65],
                    pts[po],
                    start=(akt == 0), stop=(akt == ankt - 1),
                )
            if akt == ankt - 1:
                # chunk fully accumulated: evict now (frees PSUM), defer
                # the denominator broadcast + normalize
                aqs = slice(aqc * 512, (aqc + 1) * 512)
                avs = []
                for po in range(2):
                    av = r_pool.tile([65, 512], F32, tag="avsb", name="avsb")
                    nc.vector.tensor_copy(av[:], pavs[aqc][po][:])
                    avs.append(av)
                del pavs[aqc]
                pending.append(mk_norm(0, avs[0], aqs))
                pending.append(mk_norm(1, avs[1], aqs))
                if chunk_cb is not None:
                    pending.append(lambda aqc=aqc: chunk_cb(aqc))
        if i < len(steps):
            qc, kt = steps[i]
            nkt = 4 * (qc + 1)
            # quota nkt+2 per chunk, spread over kt in [2, nkt-1): the
            # boundary steps carry the cross-chunk AV drain + new S pair,
            # so fillers there would bubble the exp stream
            if 2 <= kt < nkt - 1:
                mid = nkt - 3
                quota = nkt + 2
                j = kt - 2
                amt = (quota * (j + 1)) // mid - (quota * j) // mid
                pop(amt)
    while pending:
        if tail_warm:
            # dummy matmuls keep the HAM clock governor at full rate while
            # the DVE normalize chain runs, so the projection tail is warm
            warm_mm(3)
        pending.pop(0)()
    cnt = 0
    while state["fi"] < len(fillers):
        if tail_warm and cnt % 2 == 0:
            warm_mm(1)
        cnt += 1
        fillers[state["fi"]]()
        state["fi"] += 1


def build_nc():
    nc = bacc.Bacc("TRN2", target_bir_lowering=False, debug=False)
    xT = nc.dram_tensor("xT", [C, N], BF16, kind="ExternalInput").ap()
    wqkvT = nc.dram_tensor("wqkvT", [C, 3 * HD], BF16, kind="ExternalInput").ap()
    wpT = nc.dram_tensor("wpT", [HD, C], BF16, kind="ExternalInput").ap()
    identD = nc.dram_tensor("ident", [128, 128], BF16, kind="ExternalInput").ap()
    maskmD = nc.dram_tensor("maskm", [128, 256], BF16, kind="ExternalInput").ap()
    out = nc.dram_tensor("out", [N, C], BF16, kind="ExternalOutput").ap()

    xT_r = xT.rearrange("(ct p) n -> ct p n", p=128)
    wq_r = wqkvT.rearrange("(ct p) j -> ct p j", p=128)
    wp_r = wpT.rearrange("(ht p) co -> ht p co", p=128)

    g = Ctx()
    g.nc = nc
    g.out_r = out.rearrange("(nt p) co -> nt p co", p=128)

    with tile.TileContext(nc) as tc, ExitStack() as ctx:
        const = ctx.enter_context(tc.tile_pool(name="const", bufs=1))
        qkv_pool = ctx.enter_context(tc.tile_pool(name="qkv", bufs=1))
        yT_pool = ctx.enter_context(tc.tile_pool(name="yT", bufs=1))
        v_pool = ctx.enter_context(tc.tile_pool(name="v", bufs=1))
        mask_pool = ctx.enter_context(tc.tile_pool(name="mask", bufs=1))

        g.identity = const.tile([128, 128], BF16, tag="id")
        nc.sync.dma_start(g.identity[:], identD)
        ones64f = const.tile([128, 64], F32, tag="ones64f")
        nc.vector.memset(ones64f[:], 1.0)
        g.ones64 = const.tile([128, 64], BF16, tag="ones64")
        nc.vector.tensor_copy(g.ones64[:], ones64f[:])
        g.wtmp = const.tile([128, 512], BF16, tag="wtmp")
        nc.vector.memset(g.wtmp[:], 0.0)
        # dummy broadcast preloads the Q7 IRAM kernel (~6us) while the PE
        # is still in its warm-up window
        pbscr = const.tile([64, 64], F32, tag="pbscr")
        nc.gpsimd.partition_broadcast(pbscr[:], ones64f[0:1, :])

        # q/k tiles per pair, [d-of-pair(128), N]
        q_t = [qkv_pool.tile([128, N], BF16, tag=f"q{hp}", name=f"qT{hp}")
               for hp in range(2)]
        k_t = [qkv_pool.tile([128, N], BF16, tag=f"k{hp}", name=f"kT{hp}")
               for hp in range(2)]
        g.yT = yT_pool.tile([128, 2, N], BF16, tag="yT")
        # V per pair: [k-partition, kt, 130] = [v_even |1| v_odd |1];
        # col 64/129 = ones (softmax denominator row of the AV matmul).
        g.v_sb = [v_pool.tile([128, NT, 130], BF16, tag=f"v{hp}",
                              name=f"v{hp}") for hp in range(2)]
        # The causal triangle (fl >= p, duplicated for both heads) --
        # the only mask any diagonal k-tile needs once exp is restricted
        # to the valid strip. Prepared host-side.
        g.masks = mask_pool.tile([128, 2, 128], BF16, tag="mask")
        onescol = mask_pool.tile([128, NT], F32, tag="onescol")
        nc.vector.memset(onescol[:], 1.0)
        for hp in range(2):
            nc.vector.tensor_copy(g.v_sb[hp][:, :, 64], onescol[:])
            nc.vector.tensor_copy(g.v_sb[hp][:, :, 129], onescol[:])

        wp_pool = ctx.enter_context(tc.tile_pool(name="wp", bufs=1))
        g.wp = [wp_pool.tile([128, C], BF16, tag=f"wp{ht}", name=f"wp{ht}")
                for ht in range(2)]

        with tc.tile_pool(name="vt1", bufs=1) as vt1_pool:
            vt1 = vt1_pool.tile([128, N], BF16, tag="vt1")

            with tc.tile_pool(name="x", bufs=1) as x_pool, \
                 tc.tile_pool(name="wb", bufs=1) as wb_pool:
                xt_big = x_pool.tile([128, CT, N], BF16, tag="x",
                                     name="xt_big")
                g.xt = [xt_big[:, ct, :] for ct in range(CT)]
                wb_big = wb_pool.tile([128, CT, 384], BF16, tag="wb",
                                      name="wb_big")
                wb = [wb_big[:, ct, :] for ct in range(CT)]

                # ---- Phase 1a: pair-0 qkv, chunk-major so the PE
                # stream paces exactly behind the x chunk DMAs ----
                with tc.tile_pool(name="wa", bufs=1) as wa_pool, \
                     tc.tile_pool(name="vt0", bufs=1) as vt0_pool, \
                     tc.tile_pool(name="warm", bufs=1, space="PSUM") as wm_ps, \
                     tc.tile_pool(name="mmps", bufs=3, space="PSUM") as mm_ps:
                    wa_big = wa_pool.tile([128, CT, 384], BF16, tag="wa",
                                          name="wa_big")
                    wa = [wa_big[:, ct, :] for ct in range(CT)]
                    vt0 = vt0_pool.tile([128, N], BF16, tag="vt0")
                    # HAM warm-up: N=512 dummy matmuls keep the PE busy
                    # through the input-DMA window so the clock governor
                    # reaches full rate before the real qkv stream starts.
                    wps = wm_ps.tile([128, 512], F32, tag="warm", name="wps")
                    for _ in range(9):
                        nc.tensor.matmul(wps[:], g.identity[:], g.wtmp[:],
                                         start=True, stop=True)
                    wq_p = wq_r.rearrange("ct p j -> p ct j")
                    xT_p = xT_r.rearrange("ct p n -> p ct n")
                    # consolidated loads, ordered by first use; pair-1
                    # weights (wb) are only needed in phase 2a, so they
                    # load last and never stall the pair-0 stream.
                    nc.sync.dma_start(wa_big[:, :, 0:128], wq_p[:, :, 0:128])
                    nc.sync.dma_start(xt_big[:, 0:4, 0:512], xT_p[:, 0:4, 0:512])
                    nc.sync.dma_start(xt_big[:, 4:8, 0:512], xT_p[:, 4:8, 0:512])
                    nc.sync.dma_start(wa_big[:, :, 128:384],
                                      wq_p[:, :, 128:384])
                    for nch in range(1, QCH):
                        nc.sync.dma_start(
                            xt_big[:, :, nch * 512:(nch + 1) * 512],
                            xT_p[:, :, nch * 512:(nch + 1) * 512])
                    for ht in range(2):
                        nc.sync.dma_start(g.wp[ht][:], wp_r[ht])
                    nc.sync.dma_start(
                        g.masks[:],
                        maskmD.rearrange("p (b f) -> p b f", b=2))
                    nc.sync.dma_start(wb_big[:], wq_p[:, :, 384:768])
                    for nch in range(QCH):
                        for col0, dest, dsl in ((0, q_t[0], None),
                                                (128, k_t[0], None),
                                                (256, vt0, 0)):
                            for u in _gemm_units(g, wa, col0, dest, dsl,
                                                 mm_ps, "mm", 4, mm_ps,
                                                 "tr1a", 3, "act",
                                                 nchs=[nch]):
                                u()

                # ---- Phase 2a: pair-0 attn; rest of pair-0 qkv + pair-1
                # qkv in the exp shadow ----
                with tc.tile_pool(name="p", bufs=6) as p_pool, \
                     tc.tile_pool(name="avsb", bufs=4) as r_pool, \
                     tc.tile_pool(name="bcast", bufs=2) as bc_pool, \
                     tc.tile_pool(name="sps", bufs=3, space="PSUM") as s_ps, \
                     tc.tile_pool(name="avps", bufs=1, space="PSUM") as av_ps, \
                     tc.tile_pool(name="bcps", bufs=1, space="PSUM") as bc_ps:
                    fillers = []
                    fillers += _gemm_units(g, wb, 0, q_t[1], None,
                                           bc_ps, "mm", 1, None, "", 0, "dve")
                    fillers += _gemm_units(g, wb, 128, k_t[1], None,
                                           bc_ps, "mm", 1, None, "", 0, "dve")
                    fillers += _gemm_units(g, wb, 256, vt1, None,
                                           bc_ps, "mm", 1, None, "", 0, "dve")
                    _attention_pair(g, 0, q_t[0], k_t[0], fillers,
                                    s_ps, av_ps, bc_ps, p_pool, r_pool,
                                    bc_pool)

            # ---- Phase 2b: pair-1 attention; V1 transposes + projection
            # in the exp shadow ----
            with tc.tile_pool(name="o", bufs=6) as o_pool, \
                 tc.tile_pool(name="p2", bufs=6) as p_pool, \
                 tc.tile_pool(name="avsb2", bufs=4) as r_pool, \
                 tc.tile_pool(name="bcast2", bufs=2) as bc_pool, \
                 tc.tile_pool(name="sps2", bufs=3, space="PSUM") as s_ps, \
                 tc.tile_pool(name="avps2", bufs=1, space="PSUM") as av_ps, \
                 tc.tile_pool(name="prps", bufs=1, space="PSUM") as pr_ps:
                fillers = []

                def step_cb(qc, kt):
                    # V1 transpose for the new k-tile this chunk will touch
                    # (must precede the AV matmul that reads v_sb[1]; AV
                    # for tile 4qc+kt runs 2+ steps later).
                    nt = 4 * qc + kt
                    pst = pr_ps.tile([128, 128], BF16, tag="pr", bufs=2,
                                     name="pst")
                    nc.tensor.transpose(
                        pst[:], vt1[:, nt * 128:(nt + 1) * 128],
                        g.identity[:])
                    vd = g.v_sb[1][:, nt, :]
                    nc.vector.tensor_copy(
                        vd.rearrange("p (b c) -> p b c", b=2)[:, :, 0:64],
                        pst[:].rearrange("p (b c) -> p b c", b=2))

                def chunk_cb(qc):
                    fillers.extend(_proj_units(
                        g, qc, pr_ps, o_pool,
                        evict_engine=("mixed" if qc == QCH - 1 else "dve")))

                _attention_pair(g, 1, q_t[1], k_t[1], fillers,
                                s_ps, av_ps, s_ps, p_pool, r_pool, bc_pool,
                                chunk_cb=chunk_cb, step_cb=step_cb,
                                pbc_tag="s", pbc_bufs=2,
                                tail_warm=True)

    nc.compile()
    return nc


_NC = None


def _get_nc():
    global _NC
    if _NC is None:
        _NC = build_nc()
    return _NC


def make_in_maps(x, w_qkv, w_proj):
    x = np.asarray(x, dtype=np.float32)
    w_qkv = np.asarray(w_qkv, dtype=np.float32)
    w_proj = np.asarray(w_proj, dtype=np.float32)
    bf = ml_dtypes.bfloat16
    xT = [np.ascontiguousarray(x[b].T).astype(bf) for b in range(B)]
    ident = np.eye(128, dtype=bf)
    fl = np.arange(128)[None, :]
    p = np.arange(128)[:, None]
    tri = (fl >= p).astype(bf)
    maskm = np.ascontiguousarray(
        np.repeat(tri[:, None, :], 2, axis=1).reshape(128, 256))
    in_maps = []
    for c in range(NCORES):
        b, grp = divmod(c, NCORES // B)
        # pair-major row order: [q01 | k01 | v01 | q23 | k23 | v23]
        rows = []
        for hp in range(2):
            for s in range(3):  # q, k, v blocks of w_qkv
                base = s * C + grp * HD + hp * 2 * D
                rows.append(np.arange(base, base + 2 * D))
        rows = np.concatenate(rows)
        wqkvT = np.ascontiguousarray(w_qkv[rows, :].T).astype(bf)
        wpT = np.ascontiguousarray(w_proj[:, grp * HD:(grp + 1) * HD].T).astype(bf)
        in_maps.append({"xT": xT[b], "wqkvT": wqkvT, "wpT": wpT,
                        "ident": ident, "maskm": maskm})
    return in_maps


def assemble(results, b_proj):
    b_proj = np.asarray(b_proj, dtype=np.float32)
    out = np.zeros((B, N, C), dtype=np.float32)
    for c in range(NCORES):
        b = c // (NCORES // B)
        out[b] += results[c]["out"].astype(np.float32)
    out += b_proj[None, None, :]
    return out


def kernel(x, w_qkv, w_proj, b_proj):
    nc = _get_nc()
    in_maps = make_in_maps(x, w_qkv, w_proj)
    res = run_bass_kernel_spmd(nc, in_maps, core_ids=list(range(NCORES)))
    return assemble(res.results, b_proj)


# revision 15
# speedup vs baseline: 1.0668x; 1.0104x over previous
"""Bass/Tile Trainium2 kernel for dense causal multi-head attention.

Problem: x[2,2048,1024] -> qkv (w_qkv [3072,1024]) -> 16-head causal
attention -> out proj (w_proj [1024,1024], b_proj) -> [2,2048,1024].

Sharding over 8 NeuronCores: data-parallel over batch (2) x
tensor-parallel over heads (4 groups of 4 heads). Each core computes its
768-row slice of the qkv projection, causal attention for its 4 heads,
and a partial output projection over its 256 head-dim columns. The
all-reduce after proj is realized host-side at gather time (sum of 4
partials per batch) together with the bias add.

On-core layout: activations kept transposed ([feature, seq]) so that
  * scores are computed directly as S^T = K_tile^T-stationary @ Q-moving
    (no P transposes anywhere),
  * softmax reduction over keys happens via a ones-column appended to V
    (denominator falls out of the same PE accumulation as attn@V),
  * head pairs sit at partition offsets 0/64 and their K=64 score
    matmuls run concurrently in different PE row groups.

All matmul operands are bf16 (PSUM accumulation stays fp32): on TRN2
hardware fp32/fp32r moving operands stream at 2 cycles/column while
bf16 streams at 1, so bf16 halves tensor-engine time (the bottleneck).
exp() is applied to the fp32 PSUM scores, so only the bf16 rounding of
inputs/weights/P/V (~0.4% each, mostly incoherent) reaches the output;
tolerance is 2e-2.

Scheduling: the kernel is one long pipeline against the ScalarE exp
stream (~1.1us per k-tile step). Phase 1a computes only the chunk-0/1
slices of pair 0's qkv so attention starts early; the rest of pair 0's
qkv + all of pair 1's run as fillers inside pair 0's attention, and the
output projection inside pair 1's. Chunk-boundary normalization (PSUM
accumulator evict, denominator broadcast, 1/d multiply) is deferred
into the next chunk's early steps so the PE queue never stalls behind
it.
"""

import sys
from contextlib import ExitStack

if "/opt/trn_rl_repo" not in sys.path:
    sys.path.insert(0, "/opt/trn_rl_repo")

import numpy as np
import ml_dtypes

import concourse.bass as bass
import concourse.tile as tile
from concourse import bacc, mybir
from concourse.bass_utils import run_bass_kernel_spmd

F32 = mybir.dt.float32
BF16 = mybir.dt.bfloat16
AF = mybir.ActivationFunctionType

B, N, C = 2, 2048, 1024
H_TOT, D = 16, 64
NCORES = 8
HPC = H_TOT // (NCORES // B)  # heads per core = 4
HD = HPC * D                  # 256 per-core head-dim columns
CT = C // 128                 # 8 contraction tiles
NT = N // 128                 # 16 seq tiles
QCH = N // 512                # 4 query chunks of 512
SCALE = float(D) ** -0.5


class Ctx:
    """Shared build state."""
    pass


def _gemm_units(g, w_tiles, col0, dest, dest_slice_of, mm_pool, mm_tag,
                mm_bufs, tr_pool, tr_tag, tr_bufs, evict_engine,
                nchs=range(QCH)):
    """Filler units for one [128-col j-tile] x N GEMM: per 512-query chunk,
    4 units of 2 accumulating matmuls + 1 evict unit (+ V transposes)."""
    nc = g.nc
    units = []
    for nch in nchs:
        cell = {}
        ns = slice(nch * 512, (nch + 1) * 512)

        def mk_mm(cts, nch=nch, ns=ns, cell=cell):
            def u():
                if "ps" not in cell:
                    cell["ps"] = mm_pool.tile([128, 512], F32, tag=mm_tag,
                                              bufs=mm_bufs, name="gps")
                for ct in cts:
                    nc.tensor.matmul(
                        cell["ps"][:],
                        w_tiles[ct][:, col0:col0 + 128],
                        g.xt[ct][:, ns],
                        start=(ct == 0), stop=(ct == CT - 1),
                    )
            return u

        def mk_evict(nch=nch, ns=ns, cell=cell):
            def u():
                if evict_engine == "act":
                    nc.scalar.activation(dest[:, ns], cell["ps"][:], AF.Copy)
                else:
                    nc.vector.tensor_copy(dest[:, ns], cell["ps"][:])
            return u

        units.append(mk_mm([0, 1]))
        units.append(mk_mm([2, 3]))
        units.append(mk_mm([4, 5]))
        units.append(mk_mm([6, 7]))
        units.append(mk_evict())
        if dest_slice_of is not None:
            hp = dest_slice_of
            for nt in range(4 * nch, 4 * nch + 4):
                def tr(nt=nt, hp=hp):
                    pst = tr_pool.tile([128, 128], BF16, tag=tr_tag,
                                       bufs=tr_bufs, name="pst")
                    nc.tensor.transpose(
                        pst[:], dest[:, nt * 128:(nt + 1) * 128], g.identity[:])
                    # [v_even | v_odd] -> cols {0:64, 65:129} of the pair tile
                    vd = g.v_sb[hp][:, nt, :]
                    nc.vector.tensor_copy(
                        vd.rearrange("p (b c) -> p b c", b=2)[:, :, 0:64],
                        pst[:].rearrange("p (b c) -> p b c", b=2))
                units.append(tr)
    return units


def _proj_units(g, qc, psum_pool, o_pool, evict_engine="dve"):
    """Filler units for the output projection of seq tiles in chunk qc."""
    nc = g.nc
    units = []
    for nt in range(4 * qc, 4 * qc + 4):
        cell = {}

        def mk_mm(ht, nt=nt, cell=cell):
            def u():
                if "ps" not in cell:
                    cell["ps"] = [psum_pool.tile([128, 512], F32, tag="pr",
                                                 bufs=2, name="pso")
                                  for _ in range(2)]
                for cok in range(2):
                    nc.tensor.matmul(
                        cell["ps"][cok][:],
                        g.yT[:, ht, nt * 128:(nt + 1) * 128],
                        g.wp[ht][:, cok * 512:(cok + 1) * 512],
                        start=(ht == 0), stop=(ht == 1),
                    )
            return u

        def mk_out(cok, nt=nt, cell=cell):
            def u():
                ot = o_pool.tile([128, 512], BF16, tag="ot", name="ot")
                eng = evict_engine
                if eng == "mixed":
                    eng = "act" if cok == 0 else "dve"
                if eng == "act":
                    nc.scalar.activation(ot[:], cell["ps"][cok][:], AF.Copy)
                else:
                    nc.vector.tensor_copy(ot[:], cell["ps"][cok][:])
                nc.sync.dma_start(
                    g.out_r[nt, :, cok * 512:(cok + 1) * 512], ot[:])
            return u

        units.extend([mk_mm(0), mk_mm(1), mk_out(0), mk_out(1)])
    return units


def _attention_pair(g, hp, q_t, k_t, fillers, s_ps, av_ps, bc_ps,
                    p_pool, r_pool, bc_pool, chunk_cb=None,
                    step_cb=None, pbc_tag="pbc", pbc_bufs=1,
                    tail_warm=False):
    """Causal attention for head pair hp, popping filler units into the
    exp (ScalarE) shadow of each k-tile step.

    One flat software pipeline across all (chunk, k-tile) steps: the AV
    matmul for step i issues alongside the S matmuls of step i+2 even
    across a chunk boundary, so the exp stream never bubbles while a
    chunk's accumulators drain. Chunk-boundary normalization (evict,
    denominator broadcast, 1/d multiply) is deferred a few steps into
    the following chunk for the same reason."""
    nc = g.nc
    total_steps = sum(4 * (qc + 1) + 2 for qc in range(QCH))
    state = {"fi": 0, "step": 0}

    def pop(nsteps):
        state["step"] += nsteps
        left = total_steps - state["step"]
        avail = len(fillers) - state["fi"]
        want = avail if left <= 0 else -(-avail // (left + 1)) * nsteps
        for _ in range(min(want, avail)):
            fillers[state["fi"]]()
            state["fi"] += 1

    def make_pts(qc, kt, qs):
        # both heads' S^T tiles into one 2-bank PSUM tile -> a single
        # wide exp (amortizes the ScalarE fixed overhead). For diagonal
        # k-tiles only the causally-valid query strip [128r:] is
        # exponentiated; the fully-masked prefix is memset to zero and
        # the 128-wide triangle at the strip start gets a small bf16
        # multiply. This keeps ~30% of exp work off the ScalarE, which
        # paces the whole attention pipeline.
        diag = kt >= 4 * qc
        r = kt - 4 * qc if diag else 0
        ps = s_ps.tile([128, 2, 512], F32, tag="s", bufs=2, name="pss")
        for po in range(2):
            o = 64 * po
            nc.tensor.matmul(
                ps[:, po, :],
                k_t[o:o + 64, kt * 128:(kt + 1) * 128],
                q_t[o:o + 64, qs],
                start=True, stop=True,
            )
        ptb = p_pool.tile([128, 2, 512], BF16, tag="pt", name="pt")
        if r > 0:
            nc.vector.memset(ptb[:, :, 0:128 * r], 0.0)
        nc.scalar.activation(ptb[:, :, 128 * r:], ps[:, :, 128 * r:],
                             AF.Exp, scale=SCALE)
        if diag:
            sl = slice(128 * r, 128 * r + 128)
            nc.vector.tensor_mul(ptb[:, :, sl], ptb[:, :, sl], g.masks[:])
        return [ptb[:, 0, :], ptb[:, 1, :]]

    def mk_norm(po, av, qs):
        def f():
            # hop the denominator row to partition 0 (tiny SBUF->SBUF
            # DMA), broadcast it across partitions on the (idle) GPSIMD
            # instead of a PE rank-1 matmul, then reciprocal + scale
            dn = bc_pool.tile([1, 512], F32, tag="dn", name="dn")
            nc.sync.dma_start(dn[0:1, :], av[64:65, :])
            bcd = bc_pool.tile([64, 512], F32, tag="bcd", name="bcd")
            nc.gpsimd.partition_broadcast(bcd[:], dn[0:1, :])
            bc = bc_pool.tile([64, 512], F32, tag="bc", name="bc")
            nc.vector.reciprocal_approx_fast(bc[:], bcd[:])
            nc.vector.tensor_mul(
                g.yT[64 * po:64 * po + 64, hp, qs], av[0:64, :], bc[:])
        return f

    def warm_mm(n):
        wps = av_ps.tile([128, 512], F32, tag="av0", bufs=1, name="warm2")
        for _ in range(n):
            nc.tensor.matmul(wps[:], g.identity[:], g.wtmp[:],
                             start=True, stop=True)

    steps = [(qc, kt) for qc in range(QCH) for kt in range(4 * (qc + 1))]
    pend = []      # S/exp steps awaiting their AV (global lag of 2)
    pending = []   # deferred normalize / chunk_cb closures
    pavs = {}      # live AV accumulators by chunk
    for i in range(len(steps) + 2):
        if i < len(steps):
            qc, kt = steps[i]
            qs = slice(qc * 512, (qc + 1) * 512)
            pend.append((qc, kt, make_pts(qc, kt, qs)))
            if step_cb is not None and kt < 4:
                step_cb(qc, kt)
            if pending and 2 <= kt <= 4:
                pending.pop(0)()
        if len(pend) > 2 or (i >= len(steps) and pend):
            aqc, akt, pts = pend.pop(0)
            ankt = 4 * (aqc + 1)
            if akt == 0:
                # allocate here (not at S time) so the previous chunk's
                # accumulator evict is already issued -> clean WAR rotation
                pavs[aqc] = [av_ps.tile([65, 512], F32, tag=f"av{po}",
                                        bufs=1, name=f"pav{po}")
                             for po in range(2)]
            for po in range(2):
                nc.tensor.matmul(
                    pavs[aqc][po][:],
                    g.v_sb[hp][:, akt, 65 * po:65 * po + 65],
                    pts[po],
                    start=(akt == 0), stop=(akt == ankt - 1),
                )
            if akt == ankt - 1:
                # chunk fully accumulated: evict now (frees PSUM), defer
                # the denominator broadcast + normalize
                aqs = slice(aqc * 512, (aqc + 1) * 512)
                avs = []
                for po in range(2):
                    av = r_pool.tile([65, 512], F32, tag="avsb", name="avsb")
                    nc.vector.tensor_copy(av[:], pavs[aqc][po][:])
                    avs.append(av)
                del pavs[aqc]
                pending.append(mk_norm(0, avs[0], aqs))
                pending.append(mk_norm(1, avs[1], aqs))
                if chunk_cb is not None:
                    pending.append(lambda aqc=aqc: chunk_cb(aqc))
        if i < len(steps):
            qc, kt = steps[i]
            nkt = 4 * (qc + 1)
            # quota nkt+2 per chunk, spread over kt in [2, nkt-1): the
            # boundary steps carry the cross-chunk AV drain + new S pair,
            # so fillers there would bubble the exp stream
            if 2 <= kt < nkt - 1:
                mid = nkt - 3
                quota = nkt + 2
                j = kt - 2
                amt = (quota * (j + 1)) // mid - (quota * j) // mid
                pop(amt)
    while pending:
        if tail_warm:
            # dummy matmuls keep the HAM clock governor at full rate while
            # the DVE normalize chain runs, so the projection tail is warm
            warm_mm(3)
        pending.pop(0)()
    cnt = 0
    while state["fi"] < len(fillers):
        if tail_warm and cnt % 2 == 0:
            warm_mm(1)
        cnt += 1
        fillers[state["fi"]]()
        state["fi"] += 1


def build_nc():
    nc = bacc.Bacc("TRN2", target_bir_lowering=False, debug=False)
    # x ships chunk-major ([qch, C, 512] flattened) so every 512-query
    # chunk is one fully-contiguous DMA
    xT = nc.dram_tensor("xT", [QCH * C, 512], BF16, kind="ExternalInput").ap()
    wqkvT = nc.dram_tensor("wqkvT", [C, 3 * HD], BF16, kind="ExternalInput").ap()
    wpT = nc.dram_tensor("wpT", [HD, C], BF16, kind="ExternalInput").ap()
    identD = nc.dram_tensor("ident", [128, 128], BF16, kind="ExternalInput").ap()
    maskmD = nc.dram_tensor("maskm", [128, 256], BF16, kind="ExternalInput").ap()
    out = nc.dram_tensor("out", [N, C], BF16, kind="ExternalOutput").ap()

    xT_r = xT.rearrange("(q ct p) n -> q ct p n", q=QCH, p=128)
    wq_r = wqkvT.rearrange("(ct p) j -> ct p j", p=128)
    wp_r = wpT.rearrange("(ht p) co -> ht p co", p=128)

    g = Ctx()
    g.nc = nc
    g.out_r = out.rearrange("(nt p) co -> nt p co", p=128)

    with tile.TileContext(nc) as tc, ExitStack() as ctx:
        const = ctx.enter_context(tc.tile_pool(name="const", bufs=1))
        qkv_pool = ctx.enter_context(tc.tile_pool(name="qkv", bufs=1))
        yT_pool = ctx.enter_context(tc.tile_pool(name="yT", bufs=1))
        v_pool = ctx.enter_context(tc.tile_pool(name="v", bufs=1))
        mask_pool = ctx.enter_context(tc.tile_pool(name="mask", bufs=1))

        g.identity = const.tile([128, 128], BF16, tag="id")
        nc.sync.dma_start(g.identity[:], identD)
        ones64f = const.tile([128, 64], F32, tag="ones64f")
        nc.vector.memset(ones64f[:], 1.0)
        g.ones64 = const.tile([128, 64], BF16, tag="ones64")
        nc.vector.tensor_copy(g.ones64[:], ones64f[:])
        g.wtmp = const.tile([128, 512], BF16, tag="wtmp")
        nc.vector.memset(g.wtmp[:], 0.0)
        # dummy broadcast preloads the Q7 IRAM kernel (~6us) while the PE
        # is still in its warm-up window
        pbscr = const.tile([64, 64], F32, tag="pbscr")
        nc.gpsimd.partition_broadcast(pbscr[:], ones64f[0:1, :])

        # q/k tiles per pair, [d-of-pair(128), N]
        q_t = [qkv_pool.tile([128, N], BF16, tag=f"q{hp}", name=f"qT{hp}")
               for hp in range(2)]
        k_t = [qkv_pool.tile([128, N], BF16, tag=f"k{hp}", name=f"kT{hp}")
               for hp in range(2)]
        g.yT = yT_pool.tile([128, 2, N], BF16, tag="yT")
        # V per pair: [k-partition, kt, 130] = [v_even |1| v_odd |1];
        # col 64/129 = ones (softmax denominator row of the AV matmul).
        g.v_sb = [v_pool.tile([128, NT, 130], BF16, tag=f"v{hp}",
                              name=f"v{hp}") for hp in range(2)]
        # The causal triangle (fl >= p, duplicated for both heads) --
        # the only mask any diagonal k-tile needs once exp is restricted
        # to the valid strip. Prepared host-side.
        g.masks = mask_pool.tile([128, 2, 128], BF16, tag="mask")
        onescol = mask_pool.tile([128, NT], F32, tag="onescol")
        nc.vector.memset(onescol[:], 1.0)
        for hp in range(2):
            nc.vector.tensor_copy(g.v_sb[hp][:, :, 64], onescol[:])
            nc.vector.tensor_copy(g.v_sb[hp][:, :, 129], onescol[:])

        wp_pool = ctx.enter_context(tc.tile_pool(name="wp", bufs=1))
        g.wp = [wp_pool.tile([128, C], BF16, tag=f"wp{ht}", name=f"wp{ht}")
                for ht in range(2)]

        with tc.tile_pool(name="vt1", bufs=1) as vt1_pool:
            vt1 = vt1_pool.tile([128, N], BF16, tag="vt1")

            with tc.tile_pool(name="x", bufs=1) as x_pool, \
                 tc.tile_pool(name="wb", bufs=1) as wb_pool:
                xt_big = x_pool.tile([128, CT, N], BF16, tag="x",
                                     name="xt_big")
                g.xt = [xt_big[:, ct, :] for ct in range(CT)]
                wb_big = wb_pool.tile([128, CT, 384], BF16, tag="wb",
                                      name="wb_big")
                wb = [wb_big[:, ct, :] for ct in range(CT)]

                # ---- Phase 1a: pair-0 qkv, chunk-major so the PE
                # stream paces exactly behind the x chunk DMAs ----
                with tc.tile_pool(name="wa", bufs=1) as wa_pool, \
                     tc.tile_pool(name="vt0", bufs=1) as vt0_pool, \
                     tc.tile_pool(name="warm", bufs=1, space="PSUM") as wm_ps, \
                     tc.tile_pool(name="mmps", bufs=3, space="PSUM") as mm_ps:
                    wa_big = wa_pool.tile([128, CT, 384], BF16, tag="wa",
                                          name="wa_big")
                    wa = [wa_big[:, ct, :] for ct in range(CT)]
                    vt0 = vt0_pool.tile([128, N], BF16, tag="vt0")
                    # HAM warm-up: N=512 dummy matmuls keep the PE busy
                    # through the input-DMA window so the clock governor
                    # reaches full rate before the real qkv stream starts.
                    wps = wm_ps.tile([128, 512], F32, tag="warm", name="wps")
                    for _ in range(14):
                        nc.tensor.matmul(wps[:], g.identity[:], g.wtmp[:],
                                         start=True, stop=True)
                    wq_p = wq_r.rearrange("ct p j -> p ct j")
                    xT_p = xT_r.rearrange("q ct p n -> q p ct n")
                    # consolidated loads, ordered by first use; pair-1
                    # weights (wb) are only needed in phase 2a, so they
                    # load last and never stall the pair-0 stream.
                    nc.sync.dma_start(wa_big[:, :, 0:128], wq_p[:, :, 0:128])
                    nc.sync.dma_start(xt_big[:, 0:4, 0:512], xT_p[0, :, 0:4, :])
                    nc.sync.dma_start(xt_big[:, 4:8, 0:512], xT_p[0, :, 4:8, :])
                    nc.sync.dma_start(wa_big[:, :, 128:384],
                                      wq_p[:, :, 128:384])
                    for nch in range(1, QCH):
                        nc.sync.dma_start(
                            xt_big[:, :, nch * 512:(nch + 1) * 512],
                            xT_p[nch])
                    for ht in range(2):
                        nc.sync.dma_start(g.wp[ht][:], wp_r[ht])
                    nc.sync.dma_start(
                        g.masks[:],
                        maskmD.rearrange("p (b f) -> p b f", b=2))
                    nc.sync.dma_start(wb_big[:], wq_p[:, :, 384:768])
                    for nch in range(QCH):
                        for col0, dest, dsl in ((0, q_t[0], None),
                                                (128, k_t[0], None),
                                                (256, vt0, 0)):
                            for u in _gemm_units(g, wa, col0, dest, dsl,
                                                 mm_ps, "mm", 4, mm_ps,
                                                 "tr1a", 3, "act",
                                                 nchs=[nch]):
                                u()

                # ---- Phase 2a: pair-0 attn; rest of pair-0 qkv + pair-1
                # qkv in the exp shadow ----
                with tc.tile_pool(name="p", bufs=6) as p_pool, \
                     tc.tile_pool(name="avsb", bufs=4) as r_pool, \
                     tc.tile_pool(name="bcast", bufs=2) as bc_pool, \
                     tc.tile_pool(name="sps", bufs=3, space="PSUM") as s_ps, \
                     tc.tile_pool(name="avps", bufs=1, space="PSUM") as av_ps, \
                     tc.tile_pool(name="bcps", bufs=1, space="PSUM") as bc_ps:
                    fillers = []
                    fillers += _gemm_units(g, wb, 0, q_t[1], None,
                                           bc_ps, "mm", 1, None, "", 0, "dve")
                    fillers += _gemm_units(g, wb, 128, k_t[1], None,
                                           bc_ps, "mm", 1, None, "", 0, "dve")
                    fillers += _gemm_units(g, wb, 256, vt1, None,
                                           bc_ps, "mm", 1, None, "", 0, "dve")
                    _attention_pair(g, 0, q_t[0], k_t[0], fillers,
                                    s_ps, av_ps, bc_ps, p_pool, r_pool,
                                    bc_pool)

            # ---- Phase 2b: pair-1 attention; V1 transposes + projection
            # in the exp shadow ----
            with tc.tile_pool(name="o", bufs=6) as o_pool, \
                 tc.tile_pool(name="p2", bufs=6) as p_pool, \
                 tc.tile_pool(name="avsb2", bufs=4) as r_pool, \
                 tc.tile_pool(name="bcast2", bufs=2) as bc_pool, \
                 tc.tile_pool(name="sps2", bufs=3, space="PSUM") as s_ps, \
                 tc.tile_pool(name="avps2", bufs=1, space="PSUM") as av_ps, \
                 tc.tile_pool(name="prps", bufs=1, space="PSUM") as pr_ps:
                fillers = []

                def step_cb(qc, kt):
                    # V1 transpose for the new k-tile this chunk will touch
                    # (must precede the AV matmul that reads v_sb[1]; AV
                    # for tile 4qc+kt runs 2+ steps later).
                    nt = 4 * qc + kt
                    pst = pr_ps.tile([128, 128], BF16, tag="pr", bufs=2,
                                     name="pst")
                    nc.tensor.transpose(
                        pst[:], vt1[:, nt * 128:(nt + 1) * 128],
                        g.identity[:])
                    vd = g.v_sb[1][:, nt, :]
                    nc.vector.tensor_copy(
                        vd.rearrange("p (b c) -> p b c", b=2)[:, :, 0:64],
                        pst[:].rearrange("p (b c) -> p b c", b=2))

                def chunk_cb(qc):
                    fillers.extend(_proj_units(
                        g, qc, pr_ps, o_pool,
                        evict_engine=("mixed" if qc == QCH - 1 else "dve")))

                _attention_pair(g, 1, q_t[1], k_t[1], fillers,
                                s_ps, av_ps, s_ps, p_pool, r_pool, bc_pool,
                                chunk_cb=chunk_cb, step_cb=step_cb,
                                pbc_tag="s", pbc_bufs=2,
                                tail_warm=True)

    nc.compile()
    return nc


_NC = None


def _get_nc():
    global _NC
    if _NC is None:
        _NC = build_nc()
    return _NC


def make_in_maps(x, w_qkv, w_proj):
    x = np.asarray(x, dtype=np.float32)
    w_qkv = np.asarray(w_qkv, dtype=np.float32)
    w_proj = np.asarray(w_proj, dtype=np.float32)
    bf = ml_dtypes.bfloat16
    # [C, N] -> chunk-major [QCH, C, 512] -> [QCH*C, 512]
    xT = [np.ascontiguousarray(
              x[b].T.reshape(C, QCH, 512).transpose(1, 0, 2)
          ).reshape(QCH * C, 512).astype(bf) for b in range(B)]
    ident = np.eye(128, dtype=bf)
    fl = np.arange(128)[None, :]
    p = np.arange(128)[:, None]
    tri = (fl >= p).astype(bf)
    maskm = np.ascontiguousarray(
        np.repeat(tri[:, None, :], 2, axis=1).reshape(128, 256))
    in_maps = []
    for c in range(NCORES):
        b, grp = divmod(c, NCORES // B)
        # pair-major row order: [q01 | k01 | v01 | q23 | k23 | v23]
        rows = []
        for hp in range(2):
            for s in range(3):  # q, k, v blocks of w_qkv
                base = s * C + grp * HD + hp * 2 * D
                rows.append(np.arange(base, base + 2 * D))
        rows = np.concatenate(rows)
        wqkvT = np.ascontiguousarray(w_qkv[rows, :].T).astype(bf)
        wpT = np.ascontiguousarray(w_proj[:, grp * HD:(grp + 1) * HD].T).astype(bf)
        in_maps.append({"xT": xT[b], "wqkvT": wqkvT, "wpT": wpT,
                        "ident": ident, "maskm": maskm})
    return in_maps


def assemble(results, b_proj):
    b_proj = np.asarray(b_proj, dtype=np.float32)
    out = np.zeros((B, N, C), dtype=np.float32)
    for c in range(NCORES):
        b = c // (NCORES // B)
        out[b] += results[c]["out"].astype(np.float32)
    out += b_proj[None, None, :]
    return out


def kernel(x, w_qkv, w_proj, b_proj):
    nc = _get_nc()
    in_maps = make_in_maps(x, w_qkv, w_proj)
    res = run_bass_kernel_spmd(nc, in_maps, core_ids=list(range(NCORES)))
    return assemble(res.results, b_proj)
